# revision 1
# baseline (speedup 1.0000x reference)
"""Trainium2 Bass kernel for nn_MetaSignatureEncoder (GCN encoder with FiLM
signature conditioning), distributed over 8 NeuronCores.

Strategy (graph/data parallel, per the sharding hint):
  - Nodes are padded to NPAD = 8*49*128 = 50176 and sharded contiguously
    across the 8 cores (6272 nodes/core, 49 dst tiles of 128).
  - norm[e] = dinv[src]*dinv[dst] factors out of the message sum, so rows are
    pre-scaled once (h' = dinv * (x @ W)), edges aggregate UNWEIGHTED, and the
    dst factor is applied after aggregation.  Self-loops are a local add.
  - Phase 1: each core computes h' for its shard (PE matmul), AllGather
    replicates the full [NPAD, 512] bf16 table to every core's HBM.
  - Pass 1 (edges): per dst-tile of 128 nodes, dma_gather fetches the h' rows
    of all in-edges (grouped per tile on the host, int16 indices, split in
    two halves of the node space because gather indices are int16), and the
    segment-sum is done on the TensorEngine: for each chunk of 128 messages a
    one-hot S matrix ([128,128], S[j,d] = 1 iff dst(msg j) == d, built by one
    DVE is_equal against an iota row) is matmul'd with the gathered rows,
    accumulating in PSUM.
  - The graph signature s = sum_n relu(agg_sig + b) is reduced over nodes
    with a mask-vector matmul (PE) and AllReduce'd (f32: the fc preacts are
    ~1e4 scale, tanh sign flips forbid bf16 here).
  - gamma/beta FiLM vectors are computed redundantly on every core with f32
    matmuls whose lhsT is s broadcast along the free axis (every output
    partition gets the same row - no partition broadcast needed).
  - Encoder: FiLM + relu + LN (bn_stats/bn_aggr) per tile, conv2 matmul via
    PE transpose, dinv scale, AllGather of the [NPAD, 128] bf16 table,
    second edge pass identical in structure, FiLM + LN epilogue, output.

kernel(**inputs) takes the FULL problem inputs and returns the FULL output.
"""
import os
import sys
import math
import numpy as np
import ml_dtypes

sys.path.insert(0, "/opt/trn_rl_repo")

from concourse import bass, bacc, tile, mybir
from concourse import bass_utils

BF16 = ml_dtypes.bfloat16
dt = mybir.dt

# ---------------------------------------------------------------- config ----


class Cfg:
    def __init__(self, NT=49, C=7, n_real=50000, n_edges=800000):
        self.NC = 8           # cores
        self.TP = 128         # partitions / dst-tile size
        self.NT = NT          # dst tiles per core
        self.SPLITS = 3       # src-space segments (int16 idx + <=1024 descs/call)
        self.C = C            # gather chunks per segment per tile (128 msgs each)
        self.IN_CH = 256
        self.HID = 256
        self.OUT = 128
        self.FUSED = self.HID + self.HID   # sig(256) | conv1(256)
        self.KX = self.IN_CH // 128        # K chunks for x matmul
        self.KH = self.HID // 128          # K chunks for conv2 matmul
        self.KA = 3                        # K chunks for augmented fc matmuls
        self.SHARD = self.NT * self.TP
        self.NPAD = self.NC * self.SHARD
        # src-space segment bounds (each segment < 32768 rows for int16 idx)
        self.BOUNDS = [self.NPAD * k // self.SPLITS
                       for k in range(self.SPLITS)] + [self.NPAD]
        self.n_real = n_real
        self.n_edges = n_edges
        self.LN_EPS = 1e-5


FULL = Cfg()

# ------------------------------------------------------------ host side -----


def _wrap16(vals, nrows=128):
    """dma_gather index layout: idx j at [j % 16, j // 16], replicated to all
    8 q7 core groups (rows 16k+p == row p)."""
    n = vals.shape[0]
    assert n % 16 == 0
    w = vals.reshape(n // 16, 16).T          # [16, n/16]
    return np.tile(w, (nrows // 16, 1))      # [128, n/16]


def _pmaj(vals, TP=128):
    """[NT*TP] -> [TP, NT] partition-major (tile t col, partition p row)."""
    return np.ascontiguousarray(vals.reshape(-1, TP).T)


def preprocess(edge_index, cfg):
    """Integer-only graph preprocessing -> per-core index structures."""
    src = np.asarray(edge_index[0], dtype=np.int64)
    dst = np.asarray(edge_index[1], dtype=np.int64)
    deg = np.bincount(src, minlength=cfg.NPAD).astype(np.float32) + 1.0

    CTP = cfg.C * cfg.TP
    SP = cfg.SPLITS
    per_core = []
    shard_of = dst // cfg.SHARD
    for c in range(cfg.NC):
        m = shard_of == c
        s_c = src[m]
        d_c = dst[m] - c * cfg.SHARD
        tile_of = d_c // cfg.TP
        order = np.argsort(tile_of, kind="stable")
        s_c, d_c, tile_of = s_c[order], d_c[order], tile_of[order]
        bounds = np.searchsorted(tile_of, np.arange(cfg.NT + 1))

        idx = np.zeros((cfg.NT, SP, CTP), np.int64)
        seg = -np.ones((cfg.NT, SP, CTP), np.float32)
        for t in range(cfg.NT):
            sl = slice(bounds[t], bounds[t + 1])
            s_t = s_c[sl]
            d_t = d_c[sl] - t * cfg.TP
            for k in range(SP):
                a = (s_t >= cfg.BOUNDS[k]) & (s_t < cfg.BOUNDS[k + 1])
                na = int(a.sum())
                if na > CTP:
                    raise OverflowError(f"tile overflow: {na} > {CTP}")
                # sort by src for HBM row locality during the gather
                o = np.argsort(s_t[a], kind="stable")
                idx[t, k, :na] = (s_t[a] - cfg.BOUNDS[k])[o]
                seg[t, k, :na] = d_t[a][o]

        per_core.append({
            # gather index layout [128, NT*SP*C*8] int16
            "idx": _wrap16(idx.reshape(-1)).astype(np.int16),
            # seg layout [128, NT*SP*C]: col (t*SP+k)*C+j, partition p
            "seg": np.ascontiguousarray(
                seg.reshape(cfg.NT * SP * cfg.C, cfg.TP).T).astype(BF16),
        })
    return deg, per_core


def make_in_maps(inputs, cfg):
    """Build the per-core input maps for run_bass_kernel_spmd."""
    x = np.asarray(inputs["x"], np.float32)
    deg, per_core = preprocess(np.asarray(inputs["edge_index"]), cfg)

    xp = np.zeros((cfg.NPAD, cfg.IN_CH), np.float32)
    xp[: x.shape[0]] = x

    def chunks(a, k):  # [K*128, N] -> [K, 128, N]
        return np.ascontiguousarray(a.reshape(k, 128, a.shape[1]))

    wf = np.concatenate([np.asarray(inputs["sig_conv_w"], np.float32),
                         np.asarray(inputs["conv1_w"], np.float32)], axis=1)

    def aug(w, b):  # [N, K] weight + [N] bias -> [KA, 128, N] f32 (w.T | b | 0)
        wt = np.asarray(w, np.float32).T
        a = np.zeros((cfg.KA * 128, wt.shape[1]), np.float32)
        a[: wt.shape[0]] = wt
        a[wt.shape[0]] = np.asarray(b, np.float32)
        return chunks(a, cfg.KA)

    shared = {
        "wf": chunks(wf, cfg.KX).astype(BF16),
        "w2": chunks(np.asarray(inputs["conv2_w"], np.float32),
                     cfg.KH).astype(BF16),
        "wg1": aug(inputs["fc1_w"], inputs["fc1_b"]),
        "wb1": aug(inputs["fc2_w"], inputs["fc2_b"]),
        "wg2": aug(inputs["fc3_w"], inputs["fc3_b"]),
        "wb2": aug(inputs["fc4_w"], inputs["fc4_b"]),
        "bsig": np.broadcast_to(np.asarray(inputs["sig_conv_b"], np.float32),
                                (128, cfg.HID)).copy(),
        "b1c": np.broadcast_to(np.asarray(inputs["conv1_b"], np.float32),
                               (128, cfg.HID)).copy(),
        "b2c": np.broadcast_to(np.asarray(inputs["conv2_b"], np.float32),
                               (128, cfg.OUT)).copy(),
        "iota": np.broadcast_to(np.arange(128, dtype=np.float32),
                                (128, 128)).astype(BF16).copy(),
        "ident": np.eye(128, dtype=np.float32).astype(BF16),
    }

    in_maps = []
    node_ids = np.arange(cfg.SHARD)
    for c in range(cfg.NC):
        sl = slice(c * cfg.SHARD, (c + 1) * cfg.SHARD)
        gids = node_ids + c * cfg.SHARD
        m = dict(shared)
        m["xT"] = chunks(np.ascontiguousarray(xp[sl].T), cfg.KX).astype(BF16)
        m["deg"] = _pmaj(deg[sl]).copy()
        m["sigmask"] = _pmaj((gids < cfg.n_real).astype(np.float32)).astype(BF16)
        m.update(per_core[c])
        in_maps.append(m)
    return in_maps

# --------------------------------------------------------------- builder ----


def build_program(cfg):
    nc = bacc.Bacc("TRN2", target_bir_lowering=False, debug=False,
                   num_devices=cfg.NC,
                   num_swdge_queues=2)
    f32, bf16, i16 = dt.float32, dt.bfloat16, dt.int16
    TP, NT, C = cfg.TP, cfg.NT, cfg.C
    HID, OUT, FUSED = cfg.HID, cfg.OUT, cfg.FUSED
    CTP = C * TP

    def inp(name, shape, dtype):
        return nc.dram_tensor(name, shape, dtype, kind="ExternalInput")

    xT_d = inp("xT", [cfg.KX, TP, cfg.SHARD], bf16)
    wf_d = inp("wf", [cfg.KX, TP, FUSED], bf16)
    w2_d = inp("w2", [cfg.KH, TP, OUT], bf16)
    wg1_d = inp("wg1", [cfg.KA, TP, HID], f32)
    wb1_d = inp("wb1", [cfg.KA, TP, HID], f32)
    wg2_d = inp("wg2", [cfg.KA, TP, OUT], f32)
    wb2_d = inp("wb2", [cfg.KA, TP, OUT], f32)
    bsig_d = inp("bsig", [TP, HID], f32)
    b1c_d = inp("b1c", [TP, HID], f32)
    b2c_d = inp("b2c", [TP, OUT], f32)
    iota_d = inp("iota", [TP, TP], bf16)
    ident_d = inp("ident", [TP, TP], bf16)
    deg_d = inp("deg", [TP, NT], f32)
    mask_d = inp("sigmask", [TP, NT], bf16)
    SP = cfg.SPLITS
    idx_d = inp("idx", [TP, NT * SP * C * 8], i16)
    seg_d = inp("seg", [TP, NT * SP * C], bf16)

    out_d = nc.dram_tensor("out", [cfg.SHARD, OUT], f32, kind="ExternalOutput")
    s_dbg_d = nc.dram_tensor("s_dbg", [1, HID], f32, kind="ExternalOutput")

    # internal DRAM (collective bounce buffers)
    hsh_d = nc.dram_tensor("hsh", [cfg.SHARD, FUSED], bf16)
    hfull_d = nc.dram_tensor("hfull", [cfg.NPAD, FUSED], bf16,
                             addr_space="Shared")
    tsh_d = nc.dram_tensor("tsh", [cfg.SHARD, OUT], bf16)
    tfull_d = nc.dram_tensor("tfull", [cfg.NPAD, OUT], bf16,
                             addr_space="Shared")
    sin_d = nc.dram_tensor("sin", [1, HID], f32)
    sout_d = nc.dram_tensor("sout", [1, HID], f32, addr_space="Shared")

    rg = [list(range(cfg.NC))]

    with tile.TileContext(nc) as tc:
        with (
            tc.tile_pool(name="const", bufs=1) as const,
            tc.tile_pool(name="persist", bufs=1) as persist,
            tc.tile_pool(name="xload", bufs=3) as xload,
            tc.tile_pool(name="gat", bufs=2) as gat,
            tc.tile_pool(name="sbuild", bufs=3) as sbuild,
            tc.tile_pool(name="epi", bufs=3) as epi,
            tc.tile_pool(name="small", bufs=4) as small,
            tc.tile_pool(name="one", bufs=1) as one,
            tc.tile_pool(name="ps_big", bufs=2, space="PSUM") as ps_big,
            tc.tile_pool(name="ps_sig", bufs=1, space="PSUM") as ps_sig,
            tc.tile_pool(name="ps_sm", bufs=3, space="PSUM") as ps_sm,
        ):
            # ---- load constants -------------------------------------------
            wf_sb = const.tile([TP, cfg.KX, FUSED], bf16)
            w2_sb = const.tile([TP, cfg.KH, OUT], bf16)
            nc.sync.dma_start(out=wf_sb[:], in_=wf_d.ap().transpose([1, 0, 2]))
            nc.sync.dma_start(out=w2_sb[:], in_=w2_d.ap().transpose([1, 0, 2]))
            fc_sb = {}
            for nm, d, width in (("wg1", wg1_d, HID), ("wb1", wb1_d, HID),
                                 ("wg2", wg2_d, OUT), ("wb2", wb2_d, OUT)):
                t_ = const.tile([TP, cfg.KA, width], f32, name=nm)
                nc.sync.dma_start(out=t_[:], in_=d.ap().transpose([1, 0, 2]))
                fc_sb[nm] = t_
            bsig_sb = const.tile([TP, HID], f32)
            b1c_sb = const.tile([TP, HID], f32)
            b2c_sb = const.tile([TP, OUT], f32)
            iota_sb = const.tile([TP, TP], bf16)
            ident_sb = const.tile([TP, TP], bf16)
            deg_sb = const.tile([TP, NT], f32)
            mask_sb = const.tile([TP, NT], bf16)
            idx_sb = const.tile([TP, NT * SP * C * 8], i16)
            seg_sb = const.tile([TP, NT * SP * C], bf16)
            for t_, d in ((bsig_sb, bsig_d), (b1c_sb, b1c_d), (b2c_sb, b2c_d),
                          (iota_sb, iota_d), (ident_sb, ident_d),
                          (deg_sb, deg_d), (mask_sb, mask_d),
                          (idx_sb, idx_d), (seg_sb, seg_d)):
                nc.sync.dma_start(out=t_[:], in_=d.ap())

            eps_sb = const.tile([TP, 1], f32)
            nc.vector.memset(eps_sb[:], cfg.LN_EPS)

            # dinv = 1/sqrt(deg)
            dinv_sb = const.tile([TP, NT], f32)
            nc.scalar.sqrt(dinv_sb[:], deg_sb[:])
            nc.vector.reciprocal(dinv_sb[:], dinv_sb[:])

            # persistent per-shard state
            hsh_sb = persist.tile([TP, NT, FUSED], bf16)    # h' shard
            c1agg_sb = persist.tile([TP, NT, HID], bf16)    # conv1 aggregate
            tp_sb = persist.tile([TP, NT, OUT], bf16)       # t' shard

            # ---- phase 1: h' = dinv * (x @ [Wsig|W1]), allgather ----------
            with nc.named_scope("phase1"):
                for t in range(NT):
                    xt = xload.tile([TP, cfg.KX, TP], bf16, tag="xt")
                    for k in range(cfg.KX):
                        nc.sync.dma_start(
                            out=xt[:, k, :],
                            in_=xT_d.ap()[k, :, t * TP:(t + 1) * TP])
                    ps = ps_big.tile([TP, FUSED], f32, tag="mm")
                    for k in range(cfg.KX):
                        nc.tensor.matmul(ps[:], xt[:, k, :], wf_sb[:, k, :],
                                         start=(k == 0), stop=(k == cfg.KX - 1))
                    nc.scalar.activation(hsh_sb[:, t, :], ps[:],
                                         mybir.ActivationFunctionType.Copy,
                                         scale=dinv_sb[:, t:t + 1])
                    nc.sync.dma_start(out=hsh_d.ap()[t * TP:(t + 1) * TP, :],
                                      in_=hsh_sb[:, t, :])
                nc.gpsimd.collective_compute(
                    "AllGather", mybir.AluOpType.bypass, replica_groups=rg,
                    ins=[hsh_d.ap().opt()], outs=[hfull_d.ap().opt()])

            # ---- pass 1: edge aggregation over h' -------------------------
            s_ps = ps_sig.tile([1, HID], f32)
            qctr = [0]

            def edge_tile(t, full_d, width, ps_pool, tag):
                """gather + segsum-matmul for dst-tile t; returns psum tile."""
                ps = ps_pool.tile([TP, width], f32, tag=tag)
                first = True
                for sp in range(SP):
                    table = full_d.ap()[cfg.BOUNDS[sp]:cfg.BOUNDS[sp + 1], :]
                    g = gat.tile([TP, C, width],
                                 bf16, tag=f"g{width}", name=f"g_{t}_{sp}")
                    call = t * SP + sp
                    nc.gpsimd.dma_gather(
                        out_ap=g[:],
                        in_ap=table,
                        idxs_ap=idx_sb[:, call * C * 8:(call + 1) * C * 8],
                        num_idxs=CTP,
                        num_idxs_reg=CTP,
                        elem_size=width,
                        queue_num=qctr[0] % 2,
                    )
                    qctr[0] += 1
                    S = sbuild.tile([TP, C, TP], bf16, tag="S",
                                    name=f"S_{t}_{sp}")
                    seg_col = seg_sb[:, call * C:(call + 1) * C]
                    nc.vector.tensor_tensor(
                        S[:],
                        seg_col.unsqueeze(2).to_broadcast((TP, C, TP)),
                        iota_sb[:].unsqueeze(1).to_broadcast((TP, C, TP)),
                        mybir.AluOpType.is_equal)
                    for k in range(C):
                        nc.tensor.matmul(ps[:], S[:, k, :], g[:, k, :],
                                         start=first,
                                         stop=(sp == SP - 1 and k == C - 1))
                        first = False
                return ps

            with nc.named_scope("pass1"):
                for t in range(NT):
                    ps = edge_tile(t, hfull_d, FUSED, ps_big, "mm")
                    dv = dinv_sb[:, t:t + 1]
                    # sig half: relu((psum + selfloop)*dinv + bsig); mask-sum
                    sig_f = epi.tile([TP, HID], f32, tag="sigf")
                    nc.vector.tensor_tensor(sig_f[:], ps[:, :HID],
                                            hsh_sb[:, t, :HID],
                                            mybir.AluOpType.add)
                    nc.vector.scalar_tensor_tensor(
                        sig_f[:], sig_f[:], dv, bsig_sb[:],
                        mybir.AluOpType.mult, mybir.AluOpType.add)
                    sig_b = epi.tile([TP, HID], bf16, tag="sigb")
                    nc.scalar.activation(sig_b[:], sig_f[:],
                                         mybir.ActivationFunctionType.Relu)
                    nc.tensor.matmul(s_ps[:], mask_sb[:, t:t + 1], sig_b[:],
                                     start=(t == 0), stop=(t == NT - 1))
                    # conv1 half: (psum + selfloop)*dinv -> bf16
                    c1_f = epi.tile([TP, HID], f32, tag="c1f")
                    nc.vector.tensor_tensor(c1_f[:], ps[:, HID:],
                                            hsh_sb[:, t, HID:],
                                            mybir.AluOpType.add)
                    nc.scalar.activation(c1agg_sb[:, t, :], c1_f[:],
                                         mybir.ActivationFunctionType.Copy,
                                         scale=dv)

            # ---- signature allreduce + gamma/beta -------------------------
            with nc.named_scope("signature"):
                s_sb = one.tile([1, HID], f32)
                nc.scalar.copy(s_sb[:], s_ps[:])
                nc.sync.dma_start(out=sin_d.ap(), in_=s_sb[:])
                nc.gpsimd.collective_compute(
                    "AllReduce", mybir.AluOpType.add, replica_groups=rg,
                    ins=[sin_d.ap().opt()], outs=[sout_d.ap().opt()])
                nc.sync.dma_start(out=s_dbg_d.ap(), in_=sout_d.ap())

                # load s as columns [128, KA]; aug row (=1.0) in col KA-1
                s_col = one.tile([TP, cfg.KA], f32)
                nc.vector.memset(s_col[:], 0.0)
                nc.vector.memset(s_col[0:1, cfg.KA - 1:cfg.KA], 1.0)
                nc.sync.dma_start(
                    out=s_col[:, 0:2],
                    in_=sout_d.ap().rearrange("o (c p) -> (o c) p", p=TP)
                        .transpose([1, 0]))
                s_rep = one.tile([TP, cfg.KA, TP], f32)
                for c in range(cfg.KA):
                    nc.vector.tensor_copy(
                        s_rep[:, c, :],
                        s_col[:, c:c + 1].to_broadcast((TP, TP)))

                gb_sb = {}
                for nm, width in (("wg1", HID), ("wb1", HID),
                                  ("wg2", OUT), ("wb2", OUT)):
                    ps_fc = ps_sm.tile([TP, width], f32, tag="sm", name=nm)
                    for c in range(cfg.KA):
                        nc.tensor.matmul(ps_fc[:], s_rep[:, c, :],
                                         fc_sb[nm][:, c, :],
                                         start=(c == 0), stop=(c == cfg.KA - 1))
                    gb = one.tile([TP, width], f32, name=f"gb_{nm}", tag=nm)
                    nc.scalar.activation(gb[:], ps_fc[:],
                                         mybir.ActivationFunctionType.Tanh)
                    gb_sb[nm] = gb
                # beta + conv bias
                nc.vector.tensor_tensor(gb_sb["wb1"][:], gb_sb["wb1"][:],
                                        b1c_sb[:], mybir.AluOpType.add)
                nc.vector.tensor_tensor(gb_sb["wb2"][:], gb_sb["wb2"][:],
                                        b2c_sb[:], mybir.AluOpType.add)

            # ---- encoder local: FiLM + relu + LN + conv2 matmul -----------
            def layernorm(dst_ap, src_ap, width):
                st6 = small.tile([TP, 6], f32, tag="st6", name="st6")
                mv = small.tile([TP, 2], f32, tag="mv", name="mv")
                nc.vector.bn_stats(st6[:], src_ap)
                nc.vector.bn_aggr(mv[:], st6[:])
                std = small.tile([TP, 1], f32, tag="std", name="std")
                nc.scalar.activation(std[:], mv[:, 1:2],
                                     mybir.ActivationFunctionType.Sqrt,
                                     bias=eps_sb[:, 0:1])
                rstd = small.tile([TP, 1], f32, tag="rstd", name="rstd")
                nc.vector.reciprocal(rstd[:], std[:])
                nmr = small.tile([TP, 1], f32, tag="nmr", name="nmr")
                nc.vector.scalar_tensor_tensor(
                    nmr[:], mv[:, 0:1], -1.0, rstd[:],
                    mybir.AluOpType.mult, mybir.AluOpType.mult)
                nc.scalar.activation(dst_ap, src_ap,
                                     mybir.ActivationFunctionType.Identity,
                                     bias=nmr[:, 0:1], scale=rstd[:, 0:1])

            with nc.named_scope("encoder_local"):
                for t in range(NT):
                    h_f = epi.tile([TP, HID], f32, tag="hf", name=f"h_{t}")
                    nc.vector.tensor_tensor(h_f[:], c1agg_sb[:, t, :],
                                            gb_sb["wg1"][:],
                                            mybir.AluOpType.mult)
                    nc.vector.tensor_tensor(h_f[:], h_f[:], gb_sb["wb1"][:],
                                            mybir.AluOpType.add)
                    nc.scalar.activation(h_f[:], h_f[:],
                                         mybir.ActivationFunctionType.Relu)
                    h1 = epi.tile([TP, HID], bf16, tag="h1", name=f"h1_{t}")
                    layernorm(h1[:], h_f[:], HID)
                    # transpose h1 tile and matmul with w2
                    h1T = epi.tile([TP, cfg.KH, TP], bf16, tag="h1T",
                                   name=f"h1T_{t}")
                    for c in range(cfg.KH):
                        ps_t = ps_sm.tile([TP, TP], bf16, tag="sm",
                                          name=f"tr_{t}_{c}")
                        nc.tensor.transpose(ps_t[:],
                                            h1[:, c * TP:(c + 1) * TP],
                                            ident_sb[:])
                        nc.scalar.copy(h1T[:, c, :], ps_t[:])
                    ps2 = ps_sm.tile([TP, OUT], f32, tag="sm", name=f"w2_{t}")
                    for c in range(cfg.KH):
                        nc.tensor.matmul(ps2[:], h1T[:, c, :], w2_sb[:, c, :],
                                         start=(c == 0), stop=(c == cfg.KH - 1))
                    nc.scalar.activation(tp_sb[:, t, :], ps2[:],
                                         mybir.ActivationFunctionType.Copy,
                                         scale=dinv_sb[:, t:t + 1])
                    nc.sync.dma_start(out=tsh_d.ap()[t * TP:(t + 1) * TP, :],
                                      in_=tp_sb[:, t, :])
                nc.gpsimd.collective_compute(
                    "AllGather", mybir.AluOpType.bypass, replica_groups=rg,
                    ins=[tsh_d.ap().opt()], outs=[tfull_d.ap().opt()])

            # ---- pass 2: edge aggregation over t' -------------------------
            with nc.named_scope("pass2"):
                for t in range(NT):
                    ps = edge_tile(t, tfull_d, OUT, ps_big, "mm")
                    dv = dinv_sb[:, t:t + 1]
                    o_f = epi.tile([TP, OUT], f32, tag="of", name=f"o_{t}")
                    nc.vector.tensor_tensor(o_f[:], ps[:],
                                            tp_sb[:, t, :],
                                            mybir.AluOpType.add)
                    # gamma2 * (dinv * agg) + (beta2 + b2)
                    nc.vector.scalar_tensor_tensor(
                        o_f[:], o_f[:], dv, gb_sb["wg2"][:],
                        mybir.AluOpType.mult, mybir.AluOpType.mult)
                    nc.vector.tensor_tensor(o_f[:], o_f[:], gb_sb["wb2"][:],
                                            mybir.AluOpType.add)
                    o_ln = epi.tile([TP, OUT], f32, tag="oln", name=f"ol_{t}")
                    layernorm(o_ln[:], o_f[:], OUT)
                    nc.sync.dma_start(out=out_d.ap()[t * TP:(t + 1) * TP, :],
                                      in_=o_ln[:])

    nc.compile()
    return nc

# ---------------------------------------------------------------- runner ----


_CACHE = {}


def _get_program(cfg):
    key = (cfg.NT, cfg.C)
    if key not in _CACHE:
        _CACHE[key] = build_program(cfg)
    return _CACHE[key]


def run(inputs, cfg=FULL, trace=False, **kw):
    nc = _get_program(cfg)
    in_maps = make_in_maps(inputs, cfg)
    res = bass_utils.run_bass_kernel_spmd(
        nc, in_maps, core_ids=list(range(cfg.NC)), trace=trace, **kw)
    out = np.concatenate([res.results[c]["out"] for c in range(cfg.NC)],
                         axis=0)[: cfg.n_real]
    return out.astype(np.float32), res


def kernel(**inputs):
    out, _ = run(inputs, FULL)
    return out



# revision 2
# speedup vs baseline: 1.1911x; 1.1911x over previous
"""Trainium2 Bass kernel for nn_MetaSignatureEncoder (GCN encoder with FiLM
signature conditioning), distributed over 8 NeuronCores.

Strategy (graph/data parallel, per the sharding hint):
  - Nodes are padded to NPAD = 8*49*128 = 50176 and sharded contiguously
    across the 8 cores (6272 nodes/core, 49 dst tiles of 128).
  - norm[e] = dinv[src]*dinv[dst] factors out of the message sum, so rows are
    pre-scaled once (h' = dinv * (x @ W)), edges aggregate UNWEIGHTED, and the
    dst factor is applied after aggregation.  Self-loops are a local add.
  - Phase 1: each core computes h' for its shard (PE matmul), AllGather
    replicates the full [NPAD, 512] bf16 table to every core's HBM.
  - Pass 1 (edges): per dst-tile of 128 nodes, dma_gather fetches the h' rows
    of all in-edges (grouped per tile on the host, int16 indices, node space
    split in two segments because gather indices are int16), and the
    segment-sum is done on the TensorEngine: for each chunk of 128 messages a
    one-hot S matrix ([128,128], S[j,d] = 1 iff dst(msg j) == d, built by one
    DVE is_equal against an iota row) is matmul'd with the gathered rows,
    accumulating in PSUM.  The number of 128-message chunks per (tile, seg)
    is baked into the program per problem instance (no padded descriptors).
  - The graph signature s = sum_n relu(agg_sig + b) is reduced over nodes
    with a mask-vector matmul (PE) and AllReduce'd (f32: the fc preacts are
    ~1e4 scale, tanh sign flips forbid bf16 here).
  - gamma/beta FiLM vectors are computed redundantly on every core with f32
    matmuls whose lhsT is s broadcast along the free axis.
  - Encoder: FiLM + relu + LN (bn_stats/bn_aggr) per tile, conv2 matmul via
    PE transpose, dinv scale, AllGather of the [NPAD, 128] bf16 table,
    second edge pass identical in structure, FiLM + LN epilogue, output.

kernel(**inputs) takes the FULL problem inputs and returns the FULL output.
"""
import os
import sys
import math
import numpy as np
import ml_dtypes

sys.path.insert(0, "/opt/trn_rl_repo")

from concourse import bass, bacc, tile, mybir
from concourse import bass_utils

BF16 = ml_dtypes.bfloat16
dt = mybir.dt

# ---------------------------------------------------------------- config ----


class Cfg:
    def __init__(self, NT=49, n_real=50000, n_edges=800000):
        self.NC = 8           # cores
        self.TP = 128         # partitions / dst-tile size
        self.NT = NT          # dst tiles per core
        self.SPLITS = 2       # src-space segments (int16 idx < 32768)
        self.MAXI = 1024      # max dma_gather descriptors per call
        self.IN_CH = 256
        self.HID = 256
        self.OUT = 128
        self.FUSED = self.HID + self.HID   # sig(256) | conv1(256)
        self.KX = self.IN_CH // 128        # K chunks for x matmul
        self.KH = self.HID // 128          # K chunks for conv2 matmul
        self.KA = 3                        # K chunks for augmented fc matmuls
        self.SHARD = self.NT * self.TP
        self.NPAD = self.NC * self.SHARD
        self.BOUNDS = [self.NPAD * k // self.SPLITS
                       for k in range(self.SPLITS)] + [self.NPAD]
        self.n_real = n_real
        self.n_edges = n_edges
        self.LN_EPS = 1e-5


FULL = Cfg()

# ------------------------------------------------------------ host side -----


def _wrap16(vals, nrows=128):
    """dma_gather index layout: idx j at [j % 16, j // 16], replicated to all
    8 q7 core groups (rows 16k+p == row p)."""
    n = vals.shape[0]
    assert n % 16 == 0
    w = vals.reshape(n // 16, 16).T          # [16, n/16]
    return np.tile(w, (nrows // 16, 1))      # [128, n/16]


def _pmaj(vals, TP=128):
    """[NT*TP] -> [TP, NT] partition-major (tile t col, partition p row)."""
    return np.ascontiguousarray(vals.reshape(-1, TP).T)


def preprocess(edge_index, cfg):
    """Integer-only graph preprocessing -> per-core gather structures.

    Per core: a flat list of dma_gather calls, each for one (dst-tile, seg)
    with a compile-time chunk count C_i = ceil(n_i / 128) <= MAXI/128.
    Returns (deg, per_core, calls) where calls[i] = (tile, C_i) is shared
    by all cores (max over cores, so one SPMD program fits all).
    """
    src = np.asarray(edge_index[0], dtype=np.int64)
    dst = np.asarray(edge_index[1], dtype=np.int64)
    deg = np.bincount(src, minlength=cfg.NPAD).astype(np.float32) + 1.0

    SP = cfg.SPLITS
    # group per (core, tile, seg)
    per_core_lists = []
    shard_of = dst // cfg.SHARD
    for c in range(cfg.NC):
        m = shard_of == c
        s_c = src[m]
        d_c = dst[m] - c * cfg.SHARD
        tile_of = d_c // cfg.TP
        order = np.argsort(tile_of, kind="stable")
        s_c, d_c, tile_of = s_c[order], d_c[order], tile_of[order]
        bounds = np.searchsorted(tile_of, np.arange(cfg.NT + 1))
        tiles = []
        for t in range(cfg.NT):
            sl = slice(bounds[t], bounds[t + 1])
            s_t = s_c[sl]
            d_t = d_c[sl] - t * cfg.TP
            segs = []
            for k in range(SP):
                a = (s_t >= cfg.BOUNDS[k]) & (s_t < cfg.BOUNDS[k + 1])
                o = np.argsort(s_t[a], kind="stable")
                segs.append(((s_t[a] - cfg.BOUNDS[k])[o], d_t[a][o]))
            tiles.append(segs)
        per_core_lists.append(tiles)

    # uniform call structure: per (tile, seg) C = ceil(max_c n / 128),
    # split into multiple calls if C > MAXI/128
    maxc = cfg.MAXI // cfg.TP
    calls = []      # list of (tile, seg, chunk_count)
    for t in range(cfg.NT):
        for k in range(SP):
            nmax = max(len(per_core_lists[c][t][k][0])
                       for c in range(cfg.NC))
            C = max(1, math.ceil(nmax / cfg.TP))
            while C > maxc:
                calls.append((t, k, maxc))
                C -= maxc
            calls.append((t, k, C))

    per_core = []
    for c in range(cfg.NC):
        idx_parts, seg_parts = [], []
        consumed = {}
        for (t, k, C) in calls:
            n_slots = C * cfg.TP
            s_all, d_all = per_core_lists[c][t][k]
            lo = consumed.get((t, k), 0)
            hi = min(lo + n_slots, len(s_all))
            consumed[(t, k)] = hi
            ii = np.zeros(n_slots, np.int64)
            ss = -np.ones(n_slots, np.float32)
            ii[: hi - lo] = s_all[lo:hi]
            ss[: hi - lo] = d_all[lo:hi]
            idx_parts.append(_wrap16(ii).astype(np.int16))
            seg_parts.append(np.ascontiguousarray(
                ss.reshape(C, cfg.TP).T).astype(BF16))
        per_core.append({
            "idx": np.concatenate(idx_parts, axis=1),
            "seg": np.concatenate(seg_parts, axis=1),
        })
    return deg, per_core, tuple(calls)


def make_in_maps(inputs, cfg):
    """Build the per-core input maps for run_bass_kernel_spmd."""
    x = np.asarray(inputs["x"], np.float32)
    deg, per_core, calls = preprocess(np.asarray(inputs["edge_index"]), cfg)

    xp = np.zeros((cfg.NPAD, cfg.IN_CH), np.float32)
    xp[: x.shape[0]] = x

    def chunks(a, k):  # [K*128, N] -> [K, 128, N]
        return np.ascontiguousarray(a.reshape(k, 128, a.shape[1]))

    wf = np.concatenate([np.asarray(inputs["sig_conv_w"], np.float32),
                         np.asarray(inputs["conv1_w"], np.float32)], axis=1)

    def aug(w, b):  # [N, K] weight + [N] bias -> [KA, 128, N] f32 (w.T | b | 0)
        wt = np.asarray(w, np.float32).T
        a = np.zeros((cfg.KA * 128, wt.shape[1]), np.float32)
        a[: wt.shape[0]] = wt
        a[wt.shape[0]] = np.asarray(b, np.float32)
        return chunks(a, cfg.KA)

    shared = {
        "wf": chunks(wf, cfg.KX).astype(BF16),
        "w2": chunks(np.asarray(inputs["conv2_w"], np.float32),
                     cfg.KH).astype(BF16),
        "wg1": aug(inputs["fc1_w"], inputs["fc1_b"]),
        "wb1": aug(inputs["fc2_w"], inputs["fc2_b"]),
        "wg2": aug(inputs["fc3_w"], inputs["fc3_b"]),
        "wb2": aug(inputs["fc4_w"], inputs["fc4_b"]),
        "bsig": np.broadcast_to(np.asarray(inputs["sig_conv_b"], np.float32),
                                (128, cfg.HID)).copy(),
        "b1c": np.broadcast_to(np.asarray(inputs["conv1_b"], np.float32),
                               (128, cfg.HID)).copy(),
        "b2c": np.broadcast_to(np.asarray(inputs["conv2_b"], np.float32),
                               (128, cfg.OUT)).copy(),
        "iota": np.broadcast_to(np.arange(128, dtype=np.float32),
                                (128, 128)).astype(BF16).copy(),
        "ident": np.eye(128, dtype=np.float32).astype(BF16),
    }

    in_maps = []
    node_ids = np.arange(cfg.SHARD)
    for c in range(cfg.NC):
        sl = slice(c * cfg.SHARD, (c + 1) * cfg.SHARD)
        gids = node_ids + c * cfg.SHARD
        m = dict(shared)
        m["xT"] = chunks(np.ascontiguousarray(xp[sl].T), cfg.KX).astype(BF16)
        m["deg"] = _pmaj(deg[sl]).copy()
        m["sigmask"] = _pmaj((gids < cfg.n_real).astype(np.float32)).astype(BF16)
        m.update(per_core[c])
        in_maps.append(m)
    return in_maps, calls

# --------------------------------------------------------------- builder ----


def build_program(cfg, calls):
    nc = bacc.Bacc("TRN2", target_bir_lowering=False, debug=False,
                   num_devices=cfg.NC,
                   num_swdge_queues=2)
    f32, bf16, i16 = dt.float32, dt.bfloat16, dt.int16
    TP, NT = cfg.TP, cfg.NT
    HID, OUT, FUSED = cfg.HID, cfg.OUT, cfg.FUSED
    SP = cfg.SPLITS
    TOTC = sum(C for (_, _, C) in calls)

    def inp(name, shape, dtype):
        return nc.dram_tensor(name, shape, dtype, kind="ExternalInput")

    xT_d = inp("xT", [cfg.KX, TP, cfg.SHARD], bf16)
    wf_d = inp("wf", [cfg.KX, TP, FUSED], bf16)
    w2_d = inp("w2", [cfg.KH, TP, OUT], bf16)
    wg1_d = inp("wg1", [cfg.KA, TP, HID], f32)
    wb1_d = inp("wb1", [cfg.KA, TP, HID], f32)
    wg2_d = inp("wg2", [cfg.KA, TP, OUT], f32)
    wb2_d = inp("wb2", [cfg.KA, TP, OUT], f32)
    bsig_d = inp("bsig", [TP, HID], f32)
    b1c_d = inp("b1c", [TP, HID], f32)
    b2c_d = inp("b2c", [TP, OUT], f32)
    iota_d = inp("iota", [TP, TP], bf16)
    ident_d = inp("ident", [TP, TP], bf16)
    deg_d = inp("deg", [TP, NT], f32)
    mask_d = inp("sigmask", [TP, NT], bf16)
    idx_d = inp("idx", [TP, TOTC * 8], i16)
    seg_d = inp("seg", [TP, TOTC], bf16)

    out_d = nc.dram_tensor("out", [cfg.SHARD, OUT], f32, kind="ExternalOutput")

    # internal DRAM (collective bounce buffers)
    hsh_d = nc.dram_tensor("hsh", [cfg.SHARD, FUSED], bf16)
    hfull_d = nc.dram_tensor("hfull", [cfg.NPAD, FUSED], bf16,
                             addr_space="Shared")
    tsh_d = nc.dram_tensor("tsh", [cfg.SHARD, OUT], bf16)
    tfull_d = nc.dram_tensor("tfull", [cfg.NPAD, OUT], bf16,
                             addr_space="Shared")
    sin_d = nc.dram_tensor("sin", [1, HID], f32)
    sout_d = nc.dram_tensor("sout", [1, HID], f32, addr_space="Shared")

    rg = [list(range(cfg.NC))]

    # per-call cumulative offsets into idx/seg
    call_off = []
    o = 0
    for (_, _, C) in calls:
        call_off.append(o)
        o += C

    # group calls per tile (they are consecutive by construction)
    tile_calls = {t: [] for t in range(NT)}
    for i, (t, k, C) in enumerate(calls):
        tile_calls[t].append((i, k, C))

    with tile.TileContext(nc) as tc:
        with (
            tc.tile_pool(name="const", bufs=1) as const,
            tc.tile_pool(name="persist", bufs=1) as persist,
            tc.tile_pool(name="xload", bufs=3) as xload,
            tc.tile_pool(name="gat", bufs=3) as gat,
            tc.tile_pool(name="sbuild", bufs=3) as sbuild,
            tc.tile_pool(name="epi", bufs=3) as epi,
            tc.tile_pool(name="small", bufs=4) as small,
            tc.tile_pool(name="one", bufs=1) as one,
            tc.tile_pool(name="ps_big", bufs=2, space="PSUM") as ps_big,
            tc.tile_pool(name="ps_sig", bufs=1, space="PSUM") as ps_sig,
            tc.tile_pool(name="ps_sm", bufs=3, space="PSUM") as ps_sm,
        ):
            # ---- load constants -------------------------------------------
            wf_sb = const.tile([TP, cfg.KX, FUSED], bf16)
            w2_sb = const.tile([TP, cfg.KH, OUT], bf16)
            nc.sync.dma_start(out=wf_sb[:], in_=wf_d.ap().transpose([1, 0, 2]))
            nc.sync.dma_start(out=w2_sb[:], in_=w2_d.ap().transpose([1, 0, 2]))
            fc_sb = {}
            for nm, d, width in (("wg1", wg1_d, HID), ("wb1", wb1_d, HID),
                                 ("wg2", wg2_d, OUT), ("wb2", wb2_d, OUT)):
                t_ = const.tile([TP, cfg.KA, width], f32, name=nm)
                nc.sync.dma_start(out=t_[:], in_=d.ap().transpose([1, 0, 2]))
                fc_sb[nm] = t_
            bsig_sb = const.tile([TP, HID], f32)
            b1c_sb = const.tile([TP, HID], f32)
            b2c_sb = const.tile([TP, OUT], f32)
            iota_sb = const.tile([TP, TP], bf16)
            ident_sb = const.tile([TP, TP], bf16)
            deg_sb = const.tile([TP, NT], f32)
            mask_sb = const.tile([TP, NT], bf16)
            idx_sb = const.tile([TP, TOTC * 8], i16)
            seg_sb = const.tile([TP, TOTC], bf16)
            for t_, d in ((bsig_sb, bsig_d), (b1c_sb, b1c_d), (b2c_sb, b2c_d),
                          (iota_sb, iota_d), (ident_sb, ident_d),
                          (deg_sb, deg_d), (mask_sb, mask_d),
                          (idx_sb, idx_d), (seg_sb, seg_d)):
                nc.sync.dma_start(out=t_[:], in_=d.ap())

            eps_sb = const.tile([TP, 1], f32)
            nc.vector.memset(eps_sb[:], cfg.LN_EPS)

            # dinv = 1/sqrt(deg)
            dinv_sb = const.tile([TP, NT], f32)
            nc.scalar.sqrt(dinv_sb[:], deg_sb[:])
            nc.vector.reciprocal(dinv_sb[:], dinv_sb[:])

            # persistent per-shard state
            hsh_sb = persist.tile([TP, NT, FUSED], bf16)    # h' shard
            c1agg_sb = persist.tile([TP, NT, HID], bf16)    # conv1 aggregate
            tp_sb = persist.tile([TP, NT, OUT], bf16)       # t' shard

            # ---- phase 1: h' = dinv * (x @ [Wsig|W1]), allgather ----------
            with nc.named_scope("phase1"):
                for t in range(NT):
                    xt = xload.tile([TP, cfg.KX, TP], bf16, tag="xt")
                    for k in range(cfg.KX):
                        nc.sync.dma_start(
                            out=xt[:, k, :],
                            in_=xT_d.ap()[k, :, t * TP:(t + 1) * TP])
                    ps = ps_big.tile([TP, FUSED], f32, tag="mm")
                    for k in range(cfg.KX):
                        nc.tensor.matmul(ps[:], xt[:, k, :], wf_sb[:, k, :],
                                         start=(k == 0), stop=(k == cfg.KX - 1))
                    nc.scalar.activation(hsh_sb[:, t, :], ps[:],
                                         mybir.ActivationFunctionType.Copy,
                                         scale=dinv_sb[:, t:t + 1])
                    nc.sync.dma_start(out=hsh_d.ap()[t * TP:(t + 1) * TP, :],
                                      in_=hsh_sb[:, t, :])
                nc.gpsimd.collective_compute(
                    "AllGather", mybir.AluOpType.bypass, replica_groups=rg,
                    ins=[hsh_d.ap().opt()], outs=[hfull_d.ap().opt()])

            # ---- edge aggregation machinery -------------------------------
            s_ps = ps_sig.tile([1, HID], f32)
            qctr = [0]

            def edge_tile(t, full_d, width, ps_pool, tag):
                """gather + segsum-matmul for dst-tile t; returns psum tile."""
                ps = ps_pool.tile([TP, width], f32, tag=tag)
                first = True
                ncalls = tile_calls[t]
                for (ci, k, C) in ncalls:
                    table = full_d.ap()[cfg.BOUNDS[k]:cfg.BOUNDS[k + 1], :]
                    off = call_off[ci]
                    g = gat.tile([TP, cfg.MAXI // TP, width],
                                 bf16, tag=f"g{width}", name=f"g_{t}_{ci}")
                    nc.gpsimd.dma_gather(
                        out_ap=g[:, :C, :],
                        in_ap=table,
                        idxs_ap=idx_sb[:, off * 8:(off + C) * 8],
                        num_idxs=C * TP,
                        num_idxs_reg=C * TP,
                        elem_size=width,
                        queue_num=qctr[0] % 2,
                    )
                    qctr[0] += 1
                    S = sbuild.tile([TP, cfg.MAXI // TP, TP], bf16, tag="S",
                                    name=f"S_{t}_{ci}")
                    seg_col = seg_sb[:, off:off + C]
                    nc.vector.tensor_tensor(
                        S[:, :C, :],
                        seg_col.unsqueeze(2).to_broadcast((TP, C, TP)),
                        iota_sb[:].unsqueeze(1).to_broadcast((TP, C, TP)),
                        mybir.AluOpType.is_equal)
                    for j in range(C):
                        nc.tensor.matmul(ps[:], S[:, j, :], g[:, j, :],
                                         start=first, stop=False)
                        first = False
                return ps

            def edge_close(ps, width):
                # dummy zero-matmul to close accumulation cleanly is not
                # needed: issue the last matmul with stop via re-summing a
                # zero row would waste PE; instead the callers pass stop on
                # the last chunk by reissuing -- simpler: accumulate all with
                # stop=False then read with a stopping matmul on a zero S.
                pass

            # To mark the last matmul with stop=True we need to know it at
            # issue time; easiest is to make edge_tile handle it directly.
            def edge_tile2(t, full_d, width, ps_pool, tag):
                ps = ps_pool.tile([TP, width], f32, tag=tag)
                ncalls = tile_calls[t]
                total_chunks = sum(C for (_, _, C) in ncalls)
                done = 0
                for (ci, k, C) in ncalls:
                    table = full_d.ap()[cfg.BOUNDS[k]:cfg.BOUNDS[k + 1], :]
                    off = call_off[ci]
                    g = gat.tile([TP, cfg.MAXI // TP, width],
                                 bf16, tag=f"g{width}", name=f"g_{t}_{ci}")
                    nc.gpsimd.dma_gather(
                        out_ap=g[:, :C, :],
                        in_ap=table,
                        idxs_ap=idx_sb[:, off * 8:(off + C) * 8],
                        num_idxs=C * TP,
                        num_idxs_reg=C * TP,
                        elem_size=width,
                        queue_num=qctr[0] % 2,
                    )
                    qctr[0] += 1
                    S = sbuild.tile([TP, cfg.MAXI // TP, TP], bf16, tag="S",
                                    name=f"S_{t}_{ci}")
                    seg_col = seg_sb[:, off:off + C]
                    nc.vector.tensor_tensor(
                        S[:, :C, :],
                        seg_col.unsqueeze(2).to_broadcast((TP, C, TP)),
                        iota_sb[:].unsqueeze(1).to_broadcast((TP, C, TP)),
                        mybir.AluOpType.is_equal)
                    for j in range(C):
                        done += 1
                        nc.tensor.matmul(ps[:], S[:, j, :], g[:, j, :],
                                         start=(done == 1),
                                         stop=(done == total_chunks))
                return ps

            # ---- pass 1: edge aggregation over h' -------------------------
            with nc.named_scope("pass1"):
                for t in range(NT):
                    ps = edge_tile2(t, hfull_d, FUSED, ps_big, "mm")
                    dv = dinv_sb[:, t:t + 1]
                    # sig half: relu((psum + selfloop)*dinv + bsig); mask-sum
                    sig_f = epi.tile([TP, HID], f32, tag="sigf")
                    nc.vector.tensor_tensor(sig_f[:], ps[:, :HID],
                                            hsh_sb[:, t, :HID],
                                            mybir.AluOpType.add)
                    nc.vector.scalar_tensor_tensor(
                        sig_f[:], sig_f[:], dv, bsig_sb[:],
                        mybir.AluOpType.mult, mybir.AluOpType.add)
                    sig_b = epi.tile([TP, HID], bf16, tag="sigb")
                    nc.scalar.activation(sig_b[:], sig_f[:],
                                         mybir.ActivationFunctionType.Relu)
                    nc.tensor.matmul(s_ps[:], mask_sb[:, t:t + 1], sig_b[:],
                                     start=(t == 0), stop=(t == NT - 1))
                    # conv1 half: (psum + selfloop)*dinv -> bf16
                    c1_f = epi.tile([TP, HID], f32, tag="c1f")
                    nc.vector.tensor_tensor(c1_f[:], ps[:, HID:],
                                            hsh_sb[:, t, HID:],
                                            mybir.AluOpType.add)
                    nc.scalar.activation(c1agg_sb[:, t, :], c1_f[:],
                                         mybir.ActivationFunctionType.Copy,
                                         scale=dv)

            # ---- signature allreduce + gamma/beta -------------------------
            with nc.named_scope("signature"):
                s_sb = one.tile([1, HID], f32)
                nc.scalar.copy(s_sb[:], s_ps[:])
                nc.sync.dma_start(out=sin_d.ap(), in_=s_sb[:])
                nc.gpsimd.collective_compute(
                    "AllReduce", mybir.AluOpType.add, replica_groups=rg,
                    ins=[sin_d.ap().opt()], outs=[sout_d.ap().opt()])

                # load s as columns [128, KA]; aug row (=1.0) in col KA-1
                s_col = one.tile([TP, cfg.KA], f32)
                nc.vector.memset(s_col[:], 0.0)
                nc.vector.memset(s_col[0:1, cfg.KA - 1:cfg.KA], 1.0)
                nc.sync.dma_start(
                    out=s_col[:, 0:2],
                    in_=sout_d.ap().rearrange("o (c p) -> (o c) p", p=TP)
                        .transpose([1, 0]))
                s_rep = one.tile([TP, cfg.KA, TP], f32)
                for c in range(cfg.KA):
                    nc.vector.tensor_copy(
                        s_rep[:, c, :],
                        s_col[:, c:c + 1].to_broadcast((TP, TP)))

                gb_sb = {}
                for nm, width in (("wg1", HID), ("wb1", HID),
                                  ("wg2", OUT), ("wb2", OUT)):
                    ps_fc = ps_sm.tile([TP, width], f32, tag="sm", name=nm)
                    for c in range(cfg.KA):
                        nc.tensor.matmul(ps_fc[:], s_rep[:, c, :],
                                         fc_sb[nm][:, c, :],
                                         start=(c == 0), stop=(c == cfg.KA - 1))
                    gb = one.tile([TP, width], f32, name=f"gb_{nm}", tag=nm)
                    nc.scalar.activation(gb[:], ps_fc[:],
                                         mybir.ActivationFunctionType.Tanh)
                    gb_sb[nm] = gb
                # beta + conv bias
                nc.vector.tensor_tensor(gb_sb["wb1"][:], gb_sb["wb1"][:],
                                        b1c_sb[:], mybir.AluOpType.add)
                nc.vector.tensor_tensor(gb_sb["wb2"][:], gb_sb["wb2"][:],
                                        b2c_sb[:], mybir.AluOpType.add)

            # ---- encoder local: FiLM + relu + LN + conv2 matmul -----------
            def layernorm(dst_ap, src_ap, width):
                st6 = small.tile([TP, 6], f32, tag="st6", name="st6")
                mv = small.tile([TP, 2], f32, tag="mv", name="mv")
                nc.vector.bn_stats(st6[:], src_ap)
                nc.vector.bn_aggr(mv[:], st6[:])
                std = small.tile([TP, 1], f32, tag="std", name="std")
                nc.scalar.activation(std[:], mv[:, 1:2],
                                     mybir.ActivationFunctionType.Sqrt,
                                     bias=eps_sb[:, 0:1])
                rstd = small.tile([TP, 1], f32, tag="rstd", name="rstd")
                nc.vector.reciprocal(rstd[:], std[:])
                nmr = small.tile([TP, 1], f32, tag="nmr", name="nmr")
                nc.vector.scalar_tensor_tensor(
                    nmr[:], mv[:, 0:1], -1.0, rstd[:],
                    mybir.AluOpType.mult, mybir.AluOpType.mult)
                nc.scalar.activation(dst_ap, src_ap,
                                     mybir.ActivationFunctionType.Identity,
                                     bias=nmr[:, 0:1], scale=rstd[:, 0:1])

            with nc.named_scope("encoder_local"):
                for t in range(NT):
                    h_f = epi.tile([TP, HID], f32, tag="hf", name=f"h_{t}")
                    nc.vector.tensor_tensor(h_f[:], c1agg_sb[:, t, :],
                                            gb_sb["wg1"][:],
                                            mybir.AluOpType.mult)
                    nc.vector.tensor_tensor(h_f[:], h_f[:], gb_sb["wb1"][:],
                                            mybir.AluOpType.add)
                    nc.scalar.activation(h_f[:], h_f[:],
                                         mybir.ActivationFunctionType.Relu)
                    h1 = epi.tile([TP, HID], bf16, tag="h1", name=f"h1_{t}")
                    layernorm(h1[:], h_f[:], HID)
                    # transpose h1 tile and matmul with w2
                    h1T = epi.tile([TP, cfg.KH, TP], bf16, tag="h1T",
                                   name=f"h1T_{t}")
                    for c in range(cfg.KH):
                        ps_t = ps_sm.tile([TP, TP], bf16, tag="sm",
                                          name=f"tr_{t}_{c}")
                        nc.tensor.transpose(ps_t[:],
                                            h1[:, c * TP:(c + 1) * TP],
                                            ident_sb[:])
                        nc.scalar.copy(h1T[:, c, :], ps_t[:])
                    ps2 = ps_sm.tile([TP, OUT], f32, tag="sm", name=f"w2_{t}")
                    for c in range(cfg.KH):
                        nc.tensor.matmul(ps2[:], h1T[:, c, :], w2_sb[:, c, :],
                                         start=(c == 0), stop=(c == cfg.KH - 1))
                    nc.scalar.activation(tp_sb[:, t, :], ps2[:],
                                         mybir.ActivationFunctionType.Copy,
                                         scale=dinv_sb[:, t:t + 1])
                    nc.sync.dma_start(out=tsh_d.ap()[t * TP:(t + 1) * TP, :],
                                      in_=tp_sb[:, t, :])
                nc.gpsimd.collective_compute(
                    "AllGather", mybir.AluOpType.bypass, replica_groups=rg,
                    ins=[tsh_d.ap().opt()], outs=[tfull_d.ap().opt()])

            # ---- pass 2: edge aggregation over t' -------------------------
            with nc.named_scope("pass2"):
                for t in range(NT):
                    ps = edge_tile2(t, tfull_d, OUT, ps_big, "mm")
                    dv = dinv_sb[:, t:t + 1]
                    o_f = epi.tile([TP, OUT], f32, tag="of", name=f"o_{t}")
                    nc.vector.tensor_tensor(o_f[:], ps[:],
                                            tp_sb[:, t, :],
                                            mybir.AluOpType.add)
                    # gamma2 * (dinv * agg) + (beta2 + b2)
                    nc.vector.scalar_tensor_tensor(
                        o_f[:], o_f[:], dv, gb_sb["wg2"][:],
                        mybir.AluOpType.mult, mybir.AluOpType.mult)
                    nc.vector.tensor_tensor(o_f[:], o_f[:], gb_sb["wb2"][:],
                                            mybir.AluOpType.add)
                    o_ln = epi.tile([TP, OUT], f32, tag="oln", name=f"ol_{t}")
                    layernorm(o_ln[:], o_f[:], OUT)
                    nc.sync.dma_start(out=out_d.ap()[t * TP:(t + 1) * TP, :],
                                      in_=o_ln[:])

    nc.compile()
    return nc

# ---------------------------------------------------------------- runner ----


_CACHE = {}


def _get_program(cfg, calls):
    key = (cfg.NT, calls)
    if key not in _CACHE:
        _CACHE[key] = build_program(cfg, calls)
    return _CACHE[key]


def run(inputs, cfg=FULL, trace=False, **kw):
    in_maps, calls = make_in_maps(inputs, cfg)
    nc = _get_program(cfg, calls)
    res = bass_utils.run_bass_kernel_spmd(
        nc, in_maps, core_ids=list(range(cfg.NC)), trace=trace, **kw)
    out = np.concatenate([res.results[c]["out"] for c in range(cfg.NC)],
                         axis=0)[: cfg.n_real]
    return out.astype(np.float32), res


def kernel(**inputs):
    out, _ = run(inputs, FULL)
    return out


# revision 8
# speedup vs baseline: 1.3128x; 1.1022x over previous
"""Trainium2 Bass kernel for nn_MetaSignatureEncoder (GCN encoder with FiLM
signature conditioning), distributed over 8 NeuronCores.

Strategy (graph/data parallel, per the sharding hint):
  - Nodes are padded to NPAD = 8*49*128 = 50176 and sharded contiguously
    across the 8 cores (6272 nodes/core, 49 dst tiles of 128).
  - norm[e] = dinv[src]*dinv[dst] factors out of the message sum, so rows are
    pre-scaled once (h' = dinv * (x @ W)), edges aggregate UNWEIGHTED, and the
    dst factor is applied after aggregation.  Self-loops are a local add.
  - Phase 1: each core computes h' for its shard (PE matmul), AllGather
    replicates the full [NPAD, 512] bf16 table to every core's HBM.
  - Pass 1 (edges): per dst-tile of 128 nodes, dma_gather fetches the h' rows
    of all in-edges (grouped per tile on the host, int16 indices, node space
    split in two segments because gather indices are int16), and the
    segment-sum is done on the TensorEngine: for each chunk of 128 messages a
    one-hot S matrix ([128,128], S[j,d] = 1 iff dst(msg j) == d, built by one
    DVE is_equal against an iota row) is matmul'd with the gathered rows,
    accumulating in PSUM.  The number of 128-message chunks per (tile, seg)
    is baked into the program per problem instance (no padded descriptors).
  - The graph signature s = sum_n relu(agg_sig + b) is reduced over nodes
    with a mask-vector matmul (PE) and AllReduce'd (f32: the fc preacts are
    ~1e4 scale, tanh sign flips forbid bf16 here).
  - gamma/beta FiLM vectors are computed redundantly on every core with f32
    matmuls whose lhsT is s broadcast along the free axis.
  - Encoder: FiLM + relu + LN (bn_stats/bn_aggr) per tile, conv2 matmul via
    PE transpose, dinv scale, AllGather of the [NPAD, 128] bf16 table,
    second edge pass identical in structure, FiLM + LN epilogue, output.

kernel(**inputs) takes the FULL problem inputs and returns the FULL output.
"""
import os
import sys
import math
import numpy as np
import ml_dtypes

sys.path.insert(0, "/opt/trn_rl_repo")

from concourse import bass, bacc, tile, mybir
from concourse import bass_utils

BF16 = ml_dtypes.bfloat16
dt = mybir.dt

# ---------------------------------------------------------------- config ----


class Cfg:
    def __init__(self, NT=49, n_real=50000, n_edges=800000):
        self.NC = 8           # cores
        self.TP = 128         # partitions / dst-tile size
        self.NT = NT          # dst tiles per core
        self.SPLITS = 2       # src-space segments (int16 idx < 32768)
        self.MAXI = 1024      # max dma_gather descriptors per call
        self.IN_CH = 256
        self.HID = 256
        self.OUT = 128
        self.FUSED = self.HID + self.HID   # sig(256) | conv1(256)
        self.KX = self.IN_CH // 128        # K chunks for x matmul
        self.KH = self.HID // 128          # K chunks for conv2 matmul
        self.KA = 3                        # K chunks for augmented fc matmuls
        self.SHARD = self.NT * self.TP
        self.NPAD = self.NC * self.SHARD
        self.NT_A = 25                     # tiles in AllGather half A
        self.ROWS_A = self.NT_A * self.TP  # 3200 local rows in half A
        # half-table sizes (all cores' A rows; all cores' B rows)
        self.TBL_A = self.NC * self.ROWS_A
        self.TBL_B = self.NPAD - self.TBL_A
        self.n_real = n_real
        self.n_edges = n_edges
        self.LN_EPS = 1e-5


FULL = Cfg()

# ------------------------------------------------------------ host side -----


def _wrap16(vals, nrows=128):
    """dma_gather index layout: idx j at [j % 16, j // 16], replicated to all
    8 q7 core groups (rows 16k+p == row p)."""
    n = vals.shape[0]
    assert n % 16 == 0
    w = vals.reshape(n // 16, 16).T          # [16, n/16]
    return np.tile(w, (nrows // 16, 1))      # [128, n/16]


def _pmaj(vals, TP=128):
    """[NT*TP] -> [TP, NT] partition-major (tile t col, partition p row)."""
    return np.ascontiguousarray(vals.reshape(-1, TP).T)


def preprocess(edge_index, cfg):
    """Integer-only graph preprocessing -> per-core gather structures.

    Per core: a flat list of dma_gather calls, each for one (dst-tile, seg)
    with a compile-time chunk count C_i = ceil(n_i / 128) <= MAXI/128.
    Returns (deg, per_core, calls) where calls[i] = (tile, C_i) is shared
    by all cores (max over cores, so one SPMD program fits all).
    """
    src = np.asarray(edge_index[0], dtype=np.int64)
    dst = np.asarray(edge_index[1], dtype=np.int64)
    deg = np.bincount(src, minlength=cfg.NPAD).astype(np.float32) + 1.0

    SP = cfg.SPLITS
    # Segment = which half of its owner shard a source row lives in.
    # Half A = local rows [0, ROWS_A), half B = [ROWS_A, SHARD).  The
    # AllGather runs as two collectives (A then B), so gathers against
    # table A can start while B is still in flight.
    RA, RB = cfg.ROWS_A, cfg.SHARD - cfg.ROWS_A
    s_core = src // cfg.SHARD
    s_lr = src % cfg.SHARD
    s_seg = (s_lr >= RA).astype(np.int64)
    s_rel = np.where(s_seg == 0, s_core * RA + s_lr,
                     s_core * RB + (s_lr - RA))

    # group per (core, tile, seg)
    per_core_lists = []
    shard_of = dst // cfg.SHARD
    for c in range(cfg.NC):
        m = shard_of == c
        s_rel_c, s_seg_c = s_rel[m], s_seg[m]
        d_c = dst[m] - c * cfg.SHARD
        tile_of = d_c // cfg.TP
        order = np.argsort(tile_of, kind="stable")
        s_rel_c, s_seg_c = s_rel_c[order], s_seg_c[order]
        d_c, tile_of = d_c[order], tile_of[order]
        bounds = np.searchsorted(tile_of, np.arange(cfg.NT + 1))
        tiles = []
        for t in range(cfg.NT):
            sl = slice(bounds[t], bounds[t + 1])
            s_t, g_t = s_rel_c[sl], s_seg_c[sl]
            d_t = d_c[sl] - t * cfg.TP
            segs = []
            for k in range(SP):
                a = g_t == k
                o = np.argsort(s_t[a], kind="stable")
                segs.append((s_t[a][o], d_t[a][o]))
            tiles.append(segs)
        per_core_lists.append(tiles)

    # uniform call structure: per (tile, seg) C = ceil(max_c n / 128),
    # split into multiple calls if C > MAXI/128
    maxc = cfg.MAXI // cfg.TP
    calls = []      # list of (tile, seg, chunk_count)
    for t in range(cfg.NT):
        for k in range(SP):
            nmax = max(len(per_core_lists[c][t][k][0])
                       for c in range(cfg.NC))
            C = max(1, math.ceil(nmax / cfg.TP))
            while C > maxc:
                calls.append((t, k, maxc))
                C -= maxc
            calls.append((t, k, C))

    per_core = []
    for c in range(cfg.NC):
        idx_parts, seg_parts = [], []
        consumed = {}
        for (t, k, C) in calls:
            n_slots = C * cfg.TP
            s_all, d_all = per_core_lists[c][t][k]
            lo = consumed.get((t, k), 0)
            hi = min(lo + n_slots, len(s_all))
            consumed[(t, k)] = hi
            ii = np.zeros(n_slots, np.int64)
            ss = -np.ones(n_slots, np.float32)
            ii[: hi - lo] = s_all[lo:hi]
            ss[: hi - lo] = d_all[lo:hi]
            idx_parts.append(_wrap16(ii).astype(np.int16))
            seg_parts.append(np.ascontiguousarray(
                ss.reshape(C, cfg.TP).T).astype(BF16))
        per_core.append({
            "idx": np.concatenate(idx_parts, axis=1),
            "seg": np.concatenate(seg_parts, axis=1),
        })
    return deg, per_core, tuple(calls)


def make_in_maps(inputs, cfg):
    """Build the per-core input maps for run_bass_kernel_spmd."""
    x = np.asarray(inputs["x"], np.float32)
    deg, per_core, calls = preprocess(np.asarray(inputs["edge_index"]), cfg)

    xp = np.zeros((cfg.NPAD, cfg.IN_CH), np.float32)
    xp[: x.shape[0]] = x

    def chunks(a, k):  # [K*128, N] -> [K, 128, N]
        return np.ascontiguousarray(a.reshape(k, 128, a.shape[1]))

    wf = np.concatenate([np.asarray(inputs["sig_conv_w"], np.float32),
                         np.asarray(inputs["conv1_w"], np.float32)], axis=1)

    def aug(w, b):  # [N, K] weight + [N] bias -> [KA, 128, N] f32 (w.T | b | 0)
        wt = np.asarray(w, np.float32).T
        a = np.zeros((cfg.KA * 128, wt.shape[1]), np.float32)
        a[: wt.shape[0]] = wt
        a[wt.shape[0]] = np.asarray(b, np.float32)
        return chunks(a, cfg.KA)

    shared = {
        "wf": chunks(wf, cfg.KX).astype(BF16),
        "w2": chunks(np.asarray(inputs["conv2_w"], np.float32),
                     cfg.KH).astype(BF16),
        "wg1": aug(inputs["fc1_w"], inputs["fc1_b"]),
        "wb1": aug(inputs["fc2_w"], inputs["fc2_b"]),
        "wg2": aug(inputs["fc3_w"], inputs["fc3_b"]),
        "wb2": aug(inputs["fc4_w"], inputs["fc4_b"]),
        "bsig": np.broadcast_to(np.asarray(inputs["sig_conv_b"], np.float32),
                                (128, cfg.HID)).copy(),
        "b1c": np.broadcast_to(np.asarray(inputs["conv1_b"], np.float32),
                               (128, cfg.HID)).copy(),
        "b2c": np.broadcast_to(np.asarray(inputs["conv2_b"], np.float32),
                               (128, cfg.OUT)).copy(),
        "iota": np.broadcast_to(np.arange(128, dtype=np.float32),
                                (128, 128)).astype(BF16).copy(),
        "ident": np.eye(128, dtype=np.float32).astype(BF16),
    }

    in_maps = []
    node_ids = np.arange(cfg.SHARD)
    for c in range(cfg.NC):
        sl = slice(c * cfg.SHARD, (c + 1) * cfg.SHARD)
        gids = node_ids + c * cfg.SHARD
        m = dict(shared)
        m["xT"] = chunks(np.ascontiguousarray(xp[sl].T), cfg.KX).astype(BF16)
        m["deg"] = _pmaj(deg[sl]).copy()
        m["sigmask"] = _pmaj((gids < cfg.n_real).astype(np.float32)).astype(BF16)
        m.update(per_core[c])
        in_maps.append(m)
    return in_maps, calls

# --------------------------------------------------------------- builder ----


def build_program(cfg, calls):
    nc = bacc.Bacc("TRN2", target_bir_lowering=False, debug=False,
                   num_devices=cfg.NC,
                   num_swdge_queues=2)
    f32, bf16, i16 = dt.float32, dt.bfloat16, dt.int16
    TP, NT = cfg.TP, cfg.NT
    HID, OUT, FUSED = cfg.HID, cfg.OUT, cfg.FUSED
    SP = cfg.SPLITS
    TOTC = sum(C for (_, _, C) in calls)

    def inp(name, shape, dtype):
        return nc.dram_tensor(name, shape, dtype, kind="ExternalInput")

    xT_d = inp("xT", [cfg.KX, TP, cfg.SHARD], bf16)
    wf_d = inp("wf", [cfg.KX, TP, FUSED], bf16)
    w2_d = inp("w2", [cfg.KH, TP, OUT], bf16)
    wg1_d = inp("wg1", [cfg.KA, TP, HID], f32)
    wb1_d = inp("wb1", [cfg.KA, TP, HID], f32)
    wg2_d = inp("wg2", [cfg.KA, TP, OUT], f32)
    wb2_d = inp("wb2", [cfg.KA, TP, OUT], f32)
    bsig_d = inp("bsig", [TP, HID], f32)
    b1c_d = inp("b1c", [TP, HID], f32)
    b2c_d = inp("b2c", [TP, OUT], f32)
    iota_d = inp("iota", [TP, TP], bf16)
    ident_d = inp("ident", [TP, TP], bf16)
    deg_d = inp("deg", [TP, NT], f32)
    mask_d = inp("sigmask", [TP, NT], bf16)
    idx_d = inp("idx", [TP, TOTC * 8], i16)
    seg_d = inp("seg", [TP, TOTC], bf16)

    out_d = nc.dram_tensor("out", [cfg.SHARD, OUT], f32, kind="ExternalOutput")

    # internal DRAM (collective bounce buffers, split in two halves so the
    # second AllGather can overlap gathers against the first half)
    RA, RB = cfg.ROWS_A, cfg.SHARD - cfg.ROWS_A
    hshA_d = nc.dram_tensor("hshA", [RA, FUSED], bf16)
    hshB_d = nc.dram_tensor("hshB", [RB, FUSED], bf16)
    hfullA_d = nc.dram_tensor("hfullA", [cfg.TBL_A, FUSED], bf16,
                              addr_space="Shared")
    hfullB_d = nc.dram_tensor("hfullB", [cfg.TBL_B, FUSED], bf16,
                              addr_space="Shared")
    tshA_d = nc.dram_tensor("tshA", [RA, OUT], bf16)
    tshB_d = nc.dram_tensor("tshB", [RB, OUT], bf16)
    tfullA_d = nc.dram_tensor("tfullA", [cfg.TBL_A, OUT], bf16,
                              addr_space="Shared")
    tfullB_d = nc.dram_tensor("tfullB", [cfg.TBL_B, OUT], bf16,
                              addr_space="Shared")
    stage1_d = nc.dram_tensor("stage1", [cfg.SHARD, FUSED], f32)
    stage2_d = nc.dram_tensor("stage2", [cfg.SHARD, OUT], f32)
    sin_d = nc.dram_tensor("sin", [1, HID], f32)
    sout_d = nc.dram_tensor("sout", [1, HID], f32, addr_space="Shared")

    rg = [list(range(cfg.NC))]

    # per-call cumulative offsets into idx/seg
    call_off = []
    o = 0
    for (_, _, C) in calls:
        call_off.append(o)
        o += C

    # group calls per tile (they are consecutive by construction)
    tile_calls = {t: [] for t in range(NT)}
    for i, (t, k, C) in enumerate(calls):
        tile_calls[t].append((i, k, C))

    with tile.TileContext(nc) as tc:
        with (
            tc.tile_pool(name="const", bufs=1) as const,
            tc.tile_pool(name="persist", bufs=1) as persist,
            tc.tile_pool(name="xload", bufs=3) as xload,
            tc.tile_pool(name="gat", bufs=3) as gat,
            tc.tile_pool(name="sbuild", bufs=3) as sbuild,
            tc.tile_pool(name="epi", bufs=3) as epi,
            tc.tile_pool(name="small", bufs=4) as small,
            tc.tile_pool(name="one", bufs=1) as one,
            tc.tile_pool(name="ps_big", bufs=2, space="PSUM") as ps_big,
            tc.tile_pool(name="ps_sig", bufs=1, space="PSUM") as ps_sig,
            tc.tile_pool(name="ps_sm", bufs=3, space="PSUM") as ps_sm,
        ):
            # ---- load constants -------------------------------------------
            wf_sb = const.tile([TP, cfg.KX, FUSED], bf16)
            w2_sb = const.tile([TP, cfg.KH, OUT], bf16)
            nc.sync.dma_start(out=wf_sb[:], in_=wf_d.ap().transpose([1, 0, 2]))
            nc.sync.dma_start(out=w2_sb[:], in_=w2_d.ap().transpose([1, 0, 2]))
            fc_sb = {}
            for nm, d, width in (("wg1", wg1_d, HID), ("wb1", wb1_d, HID),
                                 ("wg2", wg2_d, OUT), ("wb2", wb2_d, OUT)):
                t_ = const.tile([TP, cfg.KA, width], f32, name=nm)
                nc.sync.dma_start(out=t_[:], in_=d.ap().transpose([1, 0, 2]))
                fc_sb[nm] = t_
            bsig_sb = const.tile([TP, HID], f32)
            b1c_sb = const.tile([TP, HID], f32)
            b2c_sb = const.tile([TP, OUT], f32)
            iota_sb = const.tile([TP, TP], bf16)
            ident_sb = const.tile([TP, TP], bf16)
            deg_sb = const.tile([TP, NT], f32)
            mask_sb = const.tile([TP, NT], bf16)
            idx_sb = const.tile([TP, TOTC * 8], i16)
            seg_sb = const.tile([TP, TOTC], bf16)
            for t_, d in ((bsig_sb, bsig_d), (b1c_sb, b1c_d), (b2c_sb, b2c_d),
                          (iota_sb, iota_d), (ident_sb, ident_d),
                          (deg_sb, deg_d), (mask_sb, mask_d),
                          (idx_sb, idx_d), (seg_sb, seg_d)):
                nc.sync.dma_start(out=t_[:], in_=d.ap())

            eps_sb = const.tile([TP, 1], f32)
            nc.vector.memset(eps_sb[:], cfg.LN_EPS)

            # dinv = 1/sqrt(deg)
            dinv_sb = const.tile([TP, NT], f32)
            nc.scalar.sqrt(dinv_sb[:], deg_sb[:])
            nc.vector.reciprocal(dinv_sb[:], dinv_sb[:])

            # persistent per-shard state
            hsh_sb = persist.tile([TP, NT, FUSED], bf16)    # h' shard
            c1agg_sb = persist.tile([TP, NT, HID], bf16)    # conv1 aggregate
            tp_sb = persist.tile([TP, NT, OUT], bf16)       # t' shard

            # ---- phase 1: h' = dinv * (x @ [Wsig|W1]), allgather ----------
            NT_A = cfg.NT_A
            with nc.named_scope("phase1"):
                for t in range(NT):
                    xt = xload.tile([TP, cfg.KX, TP], bf16, tag="xt")
                    for k in range(cfg.KX):
                        nc.sync.dma_start(
                            out=xt[:, k, :],
                            in_=xT_d.ap()[k, :, t * TP:(t + 1) * TP])
                    ps = ps_big.tile([TP, FUSED], f32, tag="mm")
                    for k in range(cfg.KX):
                        nc.tensor.matmul(ps[:], xt[:, k, :], wf_sb[:, k, :],
                                         start=(k == 0), stop=(k == cfg.KX - 1))
                    nc.scalar.activation(hsh_sb[:, t, :], ps[:],
                                         mybir.ActivationFunctionType.Copy,
                                         scale=dinv_sb[:, t:t + 1])
                    if t < NT_A:
                        dst_ap = hshA_d.ap()[t * TP:(t + 1) * TP, :]
                    else:
                        dst_ap = hshB_d.ap()[(t - NT_A) * TP:
                                             (t - NT_A + 1) * TP, :]
                    nc.sync.dma_start(out=dst_ap, in_=hsh_sb[:, t, :])
                    if t == NT_A - 1:
                        nc.gpsimd.collective_compute(
                            "AllGather", mybir.AluOpType.bypass,
                            replica_groups=rg,
                            ins=[hshA_d.ap().opt()],
                            outs=[hfullA_d.ap().opt()])
                nc.gpsimd.collective_compute(
                    "AllGather", mybir.AluOpType.bypass, replica_groups=rg,
                    ins=[hshB_d.ap().opt()], outs=[hfullB_d.ap().opt()])

            # ---- edge aggregation machinery -------------------------------
            s_ps = ps_sig.tile([1, HID], f32)
            qctr = [0]

            def seg_gather(t, k, tblA, tblB, width, ps_pool, tag):
                """gather + segsum-matmul over segment k of dst-tile t."""
                ps = ps_pool.tile([TP, width], f32, tag=tag,
                                  name=f"ps{tag}_{t}_{k}")
                ncalls = [c for c in tile_calls[t] if c[1] == k]
                total = sum(C for (_, _, C) in ncalls)
                done = 0
                for (ci, kk, C) in ncalls:
                    table = (tblA if kk == 0 else tblB).ap()
                    off = call_off[ci]
                    g = gat.tile([TP, cfg.MAXI // TP, width],
                                 bf16, tag=f"g{width}", name=f"g_{t}_{ci}")
                    nc.gpsimd.dma_gather(
                        out_ap=g[:, :C, :],
                        in_ap=table,
                        idxs_ap=idx_sb[:, off * 8:(off + C) * 8],
                        num_idxs=C * TP,
                        num_idxs_reg=C * TP,
                        elem_size=width,
                        queue_num=qctr[0] % 2,
                    )
                    qctr[0] += 1
                    S = sbuild.tile([TP, cfg.MAXI // TP, TP], bf16, tag="S",
                                    name=f"S_{t}_{ci}")
                    seg_col = seg_sb[:, off:off + C]
                    nc.vector.tensor_tensor(
                        S[:, :C, :],
                        seg_col.unsqueeze(2).to_broadcast((TP, C, TP)),
                        iota_sb[:].unsqueeze(1).to_broadcast((TP, C, TP)),
                        mybir.AluOpType.is_equal)
                    for j in range(C):
                        done += 1
                        nc.tensor.matmul(ps[:], S[:, j, :], g[:, j, :],
                                         start=(done == 1),
                                         stop=(done == total))
                return ps

            # ---- pass 1: edge aggregation over h' -------------------------
            # A phase: aggregate half-A edges for every tile, park the
            # partial sums in DRAM (stage1).  Runs as soon as the A
            # AllGather lands, overlapping the B AllGather.
            with nc.named_scope("pass1a"):
                for t in range(NT):
                    ps = seg_gather(t, 0, hfullA_d, hfullB_d, FUSED,
                                    ps_big, "mm")
                    stg = epi.tile([TP, FUSED], f32, tag="stg",
                                   name=f"stg_{t}")
                    nc.vector.tensor_copy(stg[:], ps[:])
                    nc.sync.dma_start(
                        out=stage1_d.ap()[t * TP:(t + 1) * TP, :],
                        in_=stg[:])
            with nc.named_scope("pass1b"):
                for t in range(NT):
                    ps = seg_gather(t, 1, hfullA_d, hfullB_d, FUSED,
                                    ps_big, "mm")
                    stg = epi.tile([TP, FUSED], f32, tag="stg",
                                   name=f"stgi_{t}")
                    nc.sync.dma_start(
                        out=stg[:],
                        in_=stage1_d.ap()[t * TP:(t + 1) * TP, :])
                    dv = dinv_sb[:, t:t + 1]
                    tot_f = epi.tile([TP, FUSED], f32, tag="totf",
                                     name=f"tot_{t}")
                    nc.vector.tensor_tensor(tot_f[:], ps[:], stg[:],
                                            mybir.AluOpType.add)
                    # sig half: relu((psum + selfloop)*dinv + bsig); mask-sum
                    sig_f = epi.tile([TP, HID], f32, tag="sigf")
                    nc.vector.tensor_tensor(sig_f[:], tot_f[:, :HID],
                                            hsh_sb[:, t, :HID],
                                            mybir.AluOpType.add)
                    nc.vector.scalar_tensor_tensor(
                        sig_f[:], sig_f[:], dv, bsig_sb[:],
                        mybir.AluOpType.mult, mybir.AluOpType.add)
                    sig_b = epi.tile([TP, HID], bf16, tag="sigb")
                    nc.scalar.activation(sig_b[:], sig_f[:],
                                         mybir.ActivationFunctionType.Relu)
                    nc.tensor.matmul(s_ps[:], mask_sb[:, t:t + 1], sig_b[:],
                                     start=(t == 0), stop=(t == NT - 1))
                    # conv1 half: (psum + selfloop)*dinv -> bf16
                    c1_f = epi.tile([TP, HID], f32, tag="c1f")
                    nc.vector.tensor_tensor(c1_f[:], tot_f[:, HID:],
                                            hsh_sb[:, t, HID:],
                                            mybir.AluOpType.add)
                    nc.scalar.activation(c1agg_sb[:, t, :], c1_f[:],
                                         mybir.ActivationFunctionType.Copy,
                                         scale=dv)

            # ---- signature allreduce + gamma/beta -------------------------
            with nc.named_scope("signature"):
                s_sb = one.tile([1, HID], f32)
                nc.scalar.copy(s_sb[:], s_ps[:])
                nc.sync.dma_start(out=sin_d.ap(), in_=s_sb[:])
                nc.gpsimd.collective_compute(
                    "AllReduce", mybir.AluOpType.add, replica_groups=rg,
                    ins=[sin_d.ap().opt()], outs=[sout_d.ap().opt()])

                # load s as columns [128, KA]; aug row (=1.0) in col KA-1
                s_col = one.tile([TP, cfg.KA], f32)
                nc.vector.memset(s_col[:], 0.0)
                nc.vector.memset(s_col[0:1, cfg.KA - 1:cfg.KA], 1.0)
                nc.sync.dma_start(
                    out=s_col[:, 0:2],
                    in_=sout_d.ap().rearrange("o (c p) -> (o c) p", p=TP)
                        .transpose([1, 0]))
                s_rep = one.tile([TP, cfg.KA, TP], f32)
                for c in range(cfg.KA):
                    nc.vector.tensor_copy(
                        s_rep[:, c, :],
                        s_col[:, c:c + 1].to_broadcast((TP, TP)))

                gb_sb = {}
                for nm, width in (("wg1", HID), ("wb1", HID),
                                  ("wg2", OUT), ("wb2", OUT)):
                    ps_fc = ps_sm.tile([TP, width], f32, tag="sm", name=nm)
                    for c in range(cfg.KA):
                        nc.tensor.matmul(ps_fc[:], s_rep[:, c, :],
                                         fc_sb[nm][:, c, :],
                                         start=(c == 0), stop=(c == cfg.KA - 1))
                    gb = one.tile([TP, width], f32, name=f"gb_{nm}", tag=nm)
                    nc.scalar.activation(gb[:], ps_fc[:],
                                         mybir.ActivationFunctionType.Tanh)
                    gb_sb[nm] = gb
                # beta + conv bias
                nc.vector.tensor_tensor(gb_sb["wb1"][:], gb_sb["wb1"][:],
                                        b1c_sb[:], mybir.AluOpType.add)
                nc.vector.tensor_tensor(gb_sb["wb2"][:], gb_sb["wb2"][:],
                                        b2c_sb[:], mybir.AluOpType.add)

            # ---- encoder local: FiLM + relu + LN + conv2 matmul -----------
            def layernorm(dst_ap, src_ap, width):
                st6 = small.tile([TP, 6], f32, tag="st6", name="st6")
                mv = small.tile([TP, 2], f32, tag="mv", name="mv")
                nc.vector.bn_stats(st6[:], src_ap)
                nc.vector.bn_aggr(mv[:], st6[:])
                std = small.tile([TP, 1], f32, tag="std", name="std")
                nc.scalar.activation(std[:], mv[:, 1:2],
                                     mybir.ActivationFunctionType.Sqrt,
                                     bias=eps_sb[:, 0:1])
                rstd = small.tile([TP, 1], f32, tag="rstd", name="rstd")
                nc.vector.reciprocal(rstd[:], std[:])
                nmr = small.tile([TP, 1], f32, tag="nmr", name="nmr")
                nc.vector.scalar_tensor_tensor(
                    nmr[:], mv[:, 0:1], -1.0, rstd[:],
                    mybir.AluOpType.mult, mybir.AluOpType.mult)
                nc.scalar.activation(dst_ap, src_ap,
                                     mybir.ActivationFunctionType.Identity,
                                     bias=nmr[:, 0:1], scale=rstd[:, 0:1])

            with nc.named_scope("encoder_local"):
                for t in range(NT):
                    h_f = epi.tile([TP, HID], f32, tag="hf", name=f"h_{t}")
                    nc.vector.tensor_tensor(h_f[:], c1agg_sb[:, t, :],
                                            gb_sb["wg1"][:],
                                            mybir.AluOpType.mult)
                    nc.vector.tensor_tensor(h_f[:], h_f[:], gb_sb["wb1"][:],
                                            mybir.AluOpType.add)
                    nc.scalar.activation(h_f[:], h_f[:],
                                         mybir.ActivationFunctionType.Relu)
                    h1 = epi.tile([TP, HID], bf16, tag="h1", name=f"h1_{t}")
                    layernorm(h1[:], h_f[:], HID)
                    # transpose h1 tile and matmul with w2
                    h1T = epi.tile([TP, cfg.KH, TP], bf16, tag="h1T",
                                   name=f"h1T_{t}")
                    for c in range(cfg.KH):
                        ps_t = ps_sm.tile([TP, TP], bf16, tag="sm",
                                          name=f"tr_{t}_{c}")
                        nc.tensor.transpose(ps_t[:],
                                            h1[:, c * TP:(c + 1) * TP],
                                            ident_sb[:])
                        nc.vector.tensor_copy(h1T[:, c, :], ps_t[:])
                    ps2 = ps_sm.tile([TP, OUT], f32, tag="sm", name=f"w2_{t}")
                    for c in range(cfg.KH):
                        nc.tensor.matmul(ps2[:], h1T[:, c, :], w2_sb[:, c, :],
                                         start=(c == 0), stop=(c == cfg.KH - 1))
                    nc.scalar.activation(tp_sb[:, t, :], ps2[:],
                                         mybir.ActivationFunctionType.Copy,
                                         scale=dinv_sb[:, t:t + 1])
                    if t < NT_A:
                        dst_ap = tshA_d.ap()[t * TP:(t + 1) * TP, :]
                    else:
                        dst_ap = tshB_d.ap()[(t - NT_A) * TP:
                                             (t - NT_A + 1) * TP, :]
                    nc.sync.dma_start(out=dst_ap, in_=tp_sb[:, t, :])
                    if t == NT_A - 1:
                        nc.gpsimd.collective_compute(
                            "AllGather", mybir.AluOpType.bypass,
                            replica_groups=rg,
                            ins=[tshA_d.ap().opt()],
                            outs=[tfullA_d.ap().opt()])
                nc.gpsimd.collective_compute(
                    "AllGather", mybir.AluOpType.bypass, replica_groups=rg,
                    ins=[tshB_d.ap().opt()], outs=[tfullB_d.ap().opt()])

            # ---- pass 2: edge aggregation over t' -------------------------
            with nc.named_scope("pass2a"):
                for t in range(NT):
                    ps = seg_gather(t, 0, tfullA_d, tfullB_d, OUT,
                                    ps_big, "mm2")
                    stg = epi.tile([TP, OUT], f32, tag="stg2",
                                   name=f"st2_{t}")
                    nc.vector.tensor_copy(stg[:], ps[:])
                    nc.sync.dma_start(
                        out=stage2_d.ap()[t * TP:(t + 1) * TP, :],
                        in_=stg[:])
            with nc.named_scope("pass2b"):
                for t in range(NT):
                    ps = seg_gather(t, 1, tfullA_d, tfullB_d, OUT,
                                    ps_big, "mm2")
                    stg = epi.tile([TP, OUT], f32, tag="stg2",
                                   name=f"st2i_{t}")
                    nc.sync.dma_start(
                        out=stg[:],
                        in_=stage2_d.ap()[t * TP:(t + 1) * TP, :])
                    dv = dinv_sb[:, t:t + 1]
                    o_f = epi.tile([TP, OUT], f32, tag="of", name=f"o_{t}")
                    nc.vector.tensor_tensor(o_f[:], ps[:], stg[:],
                                            mybir.AluOpType.add)
                    nc.vector.tensor_tensor(o_f[:], o_f[:],
                                            tp_sb[:, t, :],
                                            mybir.AluOpType.add)
                    # gamma2 * (dinv * agg) + (beta2 + b2)
                    nc.vector.scalar_tensor_tensor(
                        o_f[:], o_f[:], dv, gb_sb["wg2"][:],
                        mybir.AluOpType.mult, mybir.AluOpType.mult)
                    nc.vector.tensor_tensor(o_f[:], o_f[:], gb_sb["wb2"][:],
                                            mybir.AluOpType.add)
                    o_ln = epi.tile([TP, OUT], f32, tag="oln", name=f"ol_{t}")
                    layernorm(o_ln[:], o_f[:], OUT)
                    nc.sync.dma_start(out=out_d.ap()[t * TP:(t + 1) * TP, :],
                                      in_=o_ln[:])

    nc.compile()
    return nc

# ---------------------------------------------------------------- runner ----


_CACHE = {}


def _get_program(cfg, calls):
    key = (cfg.NT, calls)
    if key not in _CACHE:
        _CACHE[key] = build_program(cfg, calls)
    return _CACHE[key]


def run(inputs, cfg=FULL, trace=False, **kw):
    in_maps, calls = make_in_maps(inputs, cfg)
    nc = _get_program(cfg, calls)
    res = bass_utils.run_bass_kernel_spmd(
        nc, in_maps, core_ids=list(range(cfg.NC)), trace=trace, **kw)
    out = np.concatenate([res.results[c]["out"] for c in range(cfg.NC)],
                         axis=0)[: cfg.n_real]
    return out.astype(np.float32), res


def kernel(**inputs):
    out, _ = run(inputs, FULL)
    return out


# revision 9
# speedup vs baseline: 1.4774x; 1.1253x over previous
"""Trainium2 Bass kernel for nn_MetaSignatureEncoder (GCN encoder with FiLM
signature conditioning), distributed over 8 NeuronCores.

Strategy (graph/data parallel, per the sharding hint):
  - Nodes are padded to NPAD = 8*49*128 = 50176 and sharded contiguously
    across the 8 cores (6272 nodes/core, 49 dst tiles of 128).
  - norm[e] = dinv[src]*dinv[dst] factors out of the message sum, so rows are
    pre-scaled once (h' = dinv * (x @ W)), edges aggregate UNWEIGHTED, and the
    dst factor is applied after aggregation.  Self-loops are a local add.
  - Phase 1: each core computes h' for its shard (PE matmul), AllGather
    replicates the full [NPAD, 512] bf16 table to every core's HBM.
  - Pass 1 (edges): per dst-tile of 128 nodes, dma_gather fetches the h' rows
    of all in-edges (grouped per tile on the host, int16 indices, node space
    split in two segments because gather indices are int16), and the
    segment-sum is done on the TensorEngine: for each chunk of 128 messages a
    one-hot S matrix ([128,128], S[j,d] = 1 iff dst(msg j) == d, built by one
    DVE is_equal against an iota row) is matmul'd with the gathered rows,
    accumulating in PSUM.  The number of 128-message chunks per (tile, seg)
    is baked into the program per problem instance (no padded descriptors).
  - The graph signature s = sum_n relu(agg_sig + b) is reduced over nodes
    with a mask-vector matmul (PE) and AllReduce'd (f32: the fc preacts are
    ~1e4 scale, tanh sign flips forbid bf16 here).
  - gamma/beta FiLM vectors are computed redundantly on every core with f32
    matmuls whose lhsT is s broadcast along the free axis.
  - Encoder: FiLM + relu + LN (bn_stats/bn_aggr) per tile, conv2 matmul via
    PE transpose, dinv scale, AllGather of the [NPAD, 128] bf16 table,
    second edge pass identical in structure, FiLM + LN epilogue, output.

kernel(**inputs) takes the FULL problem inputs and returns the FULL output.
"""
import os
import sys
import math
import numpy as np
import ml_dtypes

sys.path.insert(0, "/opt/trn_rl_repo")

from concourse import bass, bacc, tile, mybir
from concourse import bass_utils

BF16 = ml_dtypes.bfloat16
dt = mybir.dt

# ---------------------------------------------------------------- config ----


class Cfg:
    def __init__(self, NT=49, n_real=50000, n_edges=800000):
        self.NC = 8           # cores
        self.TP = 128         # partitions / dst-tile size
        self.NT = NT          # dst tiles per core
        self.SPLITS = 2       # src-space segments (int16 idx < 32768)
        self.MAXI = 1024      # max dma_gather descriptors per call
        self.IN_CH = 256
        self.HID = 256
        self.OUT = 128
        self.FUSED = self.HID + self.HID   # sig(256) | conv1(256)
        self.KX = self.IN_CH // 128        # K chunks for x matmul
        self.KH = self.HID // 128          # K chunks for conv2 matmul
        self.KA = 3                        # K chunks for augmented fc matmuls
        self.SHARD = self.NT * self.TP
        self.NPAD = self.NC * self.SHARD
        self.NT_A = 25                     # tiles in AllGather half A
        self.ROWS_A = self.NT_A * self.TP  # 3200 local rows in half A
        # half-table sizes (all cores' A rows; all cores' B rows)
        self.TBL_A = self.NC * self.ROWS_A
        self.TBL_B = self.NPAD - self.TBL_A
        self.n_real = n_real
        self.n_edges = n_edges
        self.LN_EPS = 1e-5


FULL = Cfg()

# ------------------------------------------------------------ host side -----


def _wrap16(vals, nrows=128):
    """dma_gather index layout: idx j at [j % 16, j // 16], replicated to all
    8 q7 core groups (rows 16k+p == row p)."""
    n = vals.shape[0]
    assert n % 16 == 0
    w = vals.reshape(n // 16, 16).T          # [16, n/16]
    return np.tile(w, (nrows // 16, 1))      # [128, n/16]


def _pmaj(vals, TP=128):
    """[NT*TP] -> [TP, NT] partition-major (tile t col, partition p row)."""
    return np.ascontiguousarray(vals.reshape(-1, TP).T)


def preprocess(edge_index, cfg):
    """Integer-only graph preprocessing -> per-core gather structures.

    Per core: a flat list of dma_gather calls, each for one (dst-tile, seg)
    with a compile-time chunk count C_i = ceil(n_i / 128) <= MAXI/128.
    Returns (deg, per_core, calls) where calls[i] = (tile, C_i) is shared
    by all cores (max over cores, so one SPMD program fits all).
    """
    src = np.asarray(edge_index[0], dtype=np.int64)
    dst = np.asarray(edge_index[1], dtype=np.int64)
    deg = np.bincount(src, minlength=cfg.NPAD).astype(np.float32) + 1.0

    SP = cfg.SPLITS
    # Segment = which half of its owner shard a source row lives in.
    # Half A = local rows [0, ROWS_A), half B = [ROWS_A, SHARD).  The
    # AllGather runs as two collectives (A then B), so gathers against
    # table A can start while B is still in flight.
    RA, RB = cfg.ROWS_A, cfg.SHARD - cfg.ROWS_A
    s_core = src // cfg.SHARD
    s_lr = src % cfg.SHARD
    s_seg = (s_lr >= RA).astype(np.int64)
    s_rel = np.where(s_seg == 0, s_core * RA + s_lr,
                     s_core * RB + (s_lr - RA))

    # group per (core, tile, seg)
    per_core_lists = []
    shard_of = dst // cfg.SHARD
    for c in range(cfg.NC):
        m = shard_of == c
        s_rel_c, s_seg_c = s_rel[m], s_seg[m]
        d_c = dst[m] - c * cfg.SHARD
        tile_of = d_c // cfg.TP
        order = np.argsort(tile_of, kind="stable")
        s_rel_c, s_seg_c = s_rel_c[order], s_seg_c[order]
        d_c, tile_of = d_c[order], tile_of[order]
        bounds = np.searchsorted(tile_of, np.arange(cfg.NT + 1))
        tiles = []
        for t in range(cfg.NT):
            sl = slice(bounds[t], bounds[t + 1])
            s_t, g_t = s_rel_c[sl], s_seg_c[sl]
            d_t = d_c[sl] - t * cfg.TP
            segs = []
            for k in range(SP):
                a = g_t == k
                o = np.argsort(s_t[a], kind="stable")
                segs.append((s_t[a][o], d_t[a][o]))
            tiles.append(segs)
        per_core_lists.append(tiles)

    # uniform call structure: per (tile, seg) C = ceil(max_c n / 128),
    # split into multiple calls if C > MAXI/128
    maxc = cfg.MAXI // cfg.TP
    calls = []      # list of (tile, seg, chunk_count)
    for t in range(cfg.NT):
        for k in range(SP):
            nmax = max(len(per_core_lists[c][t][k][0])
                       for c in range(cfg.NC))
            C = max(1, math.ceil(nmax / cfg.TP))
            while C > maxc:
                calls.append((t, k, maxc))
                C -= maxc
            calls.append((t, k, C))

    per_core = []
    for c in range(cfg.NC):
        idx_parts, seg_parts = [], []
        consumed = {}
        for (t, k, C) in calls:
            n_slots = C * cfg.TP
            s_all, d_all = per_core_lists[c][t][k]
            lo = consumed.get((t, k), 0)
            hi = min(lo + n_slots, len(s_all))
            consumed[(t, k)] = hi
            ii = np.zeros(n_slots, np.int64)
            ss = -np.ones(n_slots, np.float32)
            ii[: hi - lo] = s_all[lo:hi]
            ss[: hi - lo] = d_all[lo:hi]
            idx_parts.append(_wrap16(ii).astype(np.int16))
            seg_parts.append(np.ascontiguousarray(
                ss.reshape(C, cfg.TP).T).astype(BF16))
        per_core.append({
            "idx": np.concatenate(idx_parts, axis=1),
            "seg": np.concatenate(seg_parts, axis=1),
        })
    return deg, per_core, tuple(calls)


def make_in_maps(inputs, cfg):
    """Build the per-core input maps for run_bass_kernel_spmd."""
    x = np.asarray(inputs["x"], np.float32)
    deg, per_core, calls = preprocess(np.asarray(inputs["edge_index"]), cfg)

    xp = np.zeros((cfg.NPAD, cfg.IN_CH), np.float32)
    xp[: x.shape[0]] = x

    def chunks(a, k):  # [K*128, N] -> [K, 128, N]
        return np.ascontiguousarray(a.reshape(k, 128, a.shape[1]))

    wf = np.concatenate([np.asarray(inputs["sig_conv_w"], np.float32),
                         np.asarray(inputs["conv1_w"], np.float32)], axis=1)

    def aug(w, b):  # [N, K] weight + [N] bias -> [KA, 128, N] f32 (w.T | b | 0)
        wt = np.asarray(w, np.float32).T
        a = np.zeros((cfg.KA * 128, wt.shape[1]), np.float32)
        a[: wt.shape[0]] = wt
        a[wt.shape[0]] = np.asarray(b, np.float32)
        return chunks(a, cfg.KA)

    shared = {
        "wf": chunks(wf, cfg.KX).astype(BF16),
        "w2": chunks(np.asarray(inputs["conv2_w"], np.float32),
                     cfg.KH).astype(BF16),
        "wg1": aug(inputs["fc1_w"], inputs["fc1_b"]),
        "wb1": aug(inputs["fc2_w"], inputs["fc2_b"]),
        "wg2": aug(inputs["fc3_w"], inputs["fc3_b"]),
        "wb2": aug(inputs["fc4_w"], inputs["fc4_b"]),
        "bsig": np.broadcast_to(np.asarray(inputs["sig_conv_b"], np.float32),
                                (128, cfg.HID)).copy(),
        "b1c": np.broadcast_to(np.asarray(inputs["conv1_b"], np.float32),
                               (128, cfg.HID)).copy(),
        "b2c": np.broadcast_to(np.asarray(inputs["conv2_b"], np.float32),
                               (128, cfg.OUT)).copy(),
        "iota": np.broadcast_to(np.arange(128, dtype=np.float32),
                                (128, 128)).astype(BF16).copy(),
        "ident": np.eye(128, dtype=np.float32).astype(BF16),
    }

    in_maps = []
    node_ids = np.arange(cfg.SHARD)
    for c in range(cfg.NC):
        sl = slice(c * cfg.SHARD, (c + 1) * cfg.SHARD)
        gids = node_ids + c * cfg.SHARD
        m = dict(shared)
        m["xT"] = chunks(np.ascontiguousarray(xp[sl].T), cfg.KX).astype(BF16)
        m["deg"] = _pmaj(deg[sl]).copy()
        m["sigmask"] = _pmaj((gids < cfg.n_real).astype(np.float32)).astype(BF16)
        m.update(per_core[c])
        in_maps.append(m)
    return in_maps, calls

# --------------------------------------------------------------- builder ----


def build_program(cfg, calls):
    nc = bacc.Bacc("TRN2", target_bir_lowering=False, debug=False,
                   num_devices=cfg.NC,
                   num_swdge_queues=4)
    f32, bf16, i16 = dt.float32, dt.bfloat16, dt.int16
    TP, NT = cfg.TP, cfg.NT
    HID, OUT, FUSED = cfg.HID, cfg.OUT, cfg.FUSED
    SP = cfg.SPLITS
    TOTC = sum(C for (_, _, C) in calls)

    def inp(name, shape, dtype):
        return nc.dram_tensor(name, shape, dtype, kind="ExternalInput")

    xT_d = inp("xT", [cfg.KX, TP, cfg.SHARD], bf16)
    wf_d = inp("wf", [cfg.KX, TP, FUSED], bf16)
    w2_d = inp("w2", [cfg.KH, TP, OUT], bf16)
    wg1_d = inp("wg1", [cfg.KA, TP, HID], f32)
    wb1_d = inp("wb1", [cfg.KA, TP, HID], f32)
    wg2_d = inp("wg2", [cfg.KA, TP, OUT], f32)
    wb2_d = inp("wb2", [cfg.KA, TP, OUT], f32)
    bsig_d = inp("bsig", [TP, HID], f32)
    b1c_d = inp("b1c", [TP, HID], f32)
    b2c_d = inp("b2c", [TP, OUT], f32)
    iota_d = inp("iota", [TP, TP], bf16)
    ident_d = inp("ident", [TP, TP], bf16)
    deg_d = inp("deg", [TP, NT], f32)
    mask_d = inp("sigmask", [TP, NT], bf16)
    idx_d = inp("idx", [TP, TOTC * 8], i16)
    seg_d = inp("seg", [TP, TOTC], bf16)

    out_d = nc.dram_tensor("out", [cfg.SHARD, OUT], f32, kind="ExternalOutput")

    # internal DRAM (collective bounce buffers, split in two halves so the
    # second AllGather can overlap gathers against the first half)
    RA, RB = cfg.ROWS_A, cfg.SHARD - cfg.ROWS_A
    hshA_d = nc.dram_tensor("hshA", [RA, FUSED], bf16)
    hshB_d = nc.dram_tensor("hshB", [RB, FUSED], bf16)
    hfullA_d = nc.dram_tensor("hfullA", [cfg.TBL_A, FUSED], bf16,
                              addr_space="Shared")
    hfullB_d = nc.dram_tensor("hfullB", [cfg.TBL_B, FUSED], bf16,
                              addr_space="Shared")
    tshA_d = nc.dram_tensor("tshA", [RA, OUT], bf16)
    tshB_d = nc.dram_tensor("tshB", [RB, OUT], bf16)
    tfullA_d = nc.dram_tensor("tfullA", [cfg.TBL_A, OUT], bf16,
                              addr_space="Shared")
    tfullB_d = nc.dram_tensor("tfullB", [cfg.TBL_B, OUT], bf16,
                              addr_space="Shared")
    stage1_d = nc.dram_tensor("stage1", [cfg.SHARD, FUSED], f32)
    stage2_d = nc.dram_tensor("stage2", [cfg.SHARD, OUT], f32)
    sin_d = nc.dram_tensor("sin", [1, HID], f32)
    sout_d = nc.dram_tensor("sout", [1, HID], f32, addr_space="Shared")

    rg = [list(range(cfg.NC))]

    # per-call cumulative offsets into idx/seg
    call_off = []
    o = 0
    for (_, _, C) in calls:
        call_off.append(o)
        o += C

    # group calls per tile (they are consecutive by construction)
    tile_calls = {t: [] for t in range(NT)}
    for i, (t, k, C) in enumerate(calls):
        tile_calls[t].append((i, k, C))

    with tile.TileContext(nc) as tc:
        with (
            tc.tile_pool(name="const", bufs=1) as const,
            tc.tile_pool(name="persist", bufs=1) as persist,
            tc.tile_pool(name="xload", bufs=3) as xload,
            tc.tile_pool(name="gat", bufs=4) as gat,
            tc.tile_pool(name="sbuild", bufs=3) as sbuild,
            tc.tile_pool(name="epi", bufs=3) as epi,
            tc.tile_pool(name="small", bufs=4) as small,
            tc.tile_pool(name="one", bufs=1) as one,
            tc.tile_pool(name="ps_big", bufs=2, space="PSUM") as ps_big,
            tc.tile_pool(name="ps_sig", bufs=1, space="PSUM") as ps_sig,
            tc.tile_pool(name="ps_sm", bufs=3, space="PSUM") as ps_sm,
        ):
            # ---- load constants -------------------------------------------
            wf_sb = const.tile([TP, cfg.KX, FUSED], bf16)
            w2_sb = const.tile([TP, cfg.KH, OUT], bf16)
            nc.sync.dma_start(out=wf_sb[:], in_=wf_d.ap().transpose([1, 0, 2]))
            nc.sync.dma_start(out=w2_sb[:], in_=w2_d.ap().transpose([1, 0, 2]))
            fc_sb = {}
            for nm, d, width in (("wg1", wg1_d, HID), ("wb1", wb1_d, HID),
                                 ("wg2", wg2_d, OUT), ("wb2", wb2_d, OUT)):
                t_ = const.tile([TP, cfg.KA, width], f32, name=nm)
                nc.sync.dma_start(out=t_[:], in_=d.ap().transpose([1, 0, 2]))
                fc_sb[nm] = t_
            bsig_sb = const.tile([TP, HID], f32)
            b1c_sb = const.tile([TP, HID], f32)
            b2c_sb = const.tile([TP, OUT], f32)
            iota_sb = const.tile([TP, TP], bf16)
            ident_sb = const.tile([TP, TP], bf16)
            deg_sb = const.tile([TP, NT], f32)
            mask_sb = const.tile([TP, NT], bf16)
            idx_sb = const.tile([TP, TOTC * 8], i16)
            seg_sb = const.tile([TP, TOTC], bf16)
            for t_, d in ((bsig_sb, bsig_d), (b1c_sb, b1c_d), (b2c_sb, b2c_d),
                          (iota_sb, iota_d), (ident_sb, ident_d),
                          (deg_sb, deg_d), (mask_sb, mask_d),
                          (idx_sb, idx_d), (seg_sb, seg_d)):
                nc.sync.dma_start(out=t_[:], in_=d.ap())

            eps_sb = const.tile([TP, 1], f32)
            nc.vector.memset(eps_sb[:], cfg.LN_EPS)

            # dinv = 1/sqrt(deg)
            dinv_sb = const.tile([TP, NT], f32)
            nc.scalar.sqrt(dinv_sb[:], deg_sb[:])
            nc.vector.reciprocal(dinv_sb[:], dinv_sb[:])

            # persistent per-shard state
            hsh_sb = persist.tile([TP, NT, FUSED], bf16)    # h' shard
            c1agg_sb = persist.tile([TP, NT, HID], bf16)    # conv1 aggregate
            tp_sb = persist.tile([TP, NT, OUT], bf16)       # t' shard

            # ---- phase 1: h' = dinv * (x @ [Wsig|W1]), allgather ----------
            NT_A = cfg.NT_A
            with nc.named_scope("phase1"):
                for t in range(NT):
                    xt = xload.tile([TP, cfg.KX, TP], bf16, tag="xt")
                    for k in range(cfg.KX):
                        nc.sync.dma_start(
                            out=xt[:, k, :],
                            in_=xT_d.ap()[k, :, t * TP:(t + 1) * TP])
                    ps = ps_big.tile([TP, FUSED], f32, tag="mm")
                    for k in range(cfg.KX):
                        nc.tensor.matmul(ps[:], xt[:, k, :], wf_sb[:, k, :],
                                         start=(k == 0), stop=(k == cfg.KX - 1))
                    nc.scalar.activation(hsh_sb[:, t, :], ps[:],
                                         mybir.ActivationFunctionType.Copy,
                                         scale=dinv_sb[:, t:t + 1])
                    if t < NT_A:
                        dst_ap = hshA_d.ap()[t * TP:(t + 1) * TP, :]
                    else:
                        dst_ap = hshB_d.ap()[(t - NT_A) * TP:
                                             (t - NT_A + 1) * TP, :]
                    nc.sync.dma_start(out=dst_ap, in_=hsh_sb[:, t, :])
                    if t == NT_A - 1:
                        nc.gpsimd.collective_compute(
                            "AllGather", mybir.AluOpType.bypass,
                            replica_groups=rg,
                            ins=[hshA_d.ap().opt()],
                            outs=[hfullA_d.ap().opt()])
                nc.gpsimd.collective_compute(
                    "AllGather", mybir.AluOpType.bypass, replica_groups=rg,
                    ins=[hshB_d.ap().opt()], outs=[hfullB_d.ap().opt()])

            # ---- edge aggregation machinery -------------------------------
            s_ps = ps_sig.tile([1, HID], f32)
            qctr = [0]

            def seg_gather(t, k, tblA, tblB, width, ps_pool, tag):
                """gather + segsum-matmul over segment k of dst-tile t."""
                ps = ps_pool.tile([TP, width], f32, tag=tag,
                                  name=f"ps{tag}_{t}_{k}")
                ncalls = [c for c in tile_calls[t] if c[1] == k]
                total = sum(C for (_, _, C) in ncalls)
                done = 0
                for (ci, kk, C) in ncalls:
                    table = (tblA if kk == 0 else tblB).ap()
                    off = call_off[ci]
                    g = gat.tile([TP, cfg.MAXI // TP, width],
                                 bf16, tag=f"g{width}", name=f"g_{t}_{ci}")
                    nc.gpsimd.dma_gather(
                        out_ap=g[:, :C, :],
                        in_ap=table,
                        idxs_ap=idx_sb[:, off * 8:(off + C) * 8],
                        num_idxs=C * TP,
                        num_idxs_reg=C * TP,
                        elem_size=width,
                        queue_num=qctr[0] % 4,
                    )
                    qctr[0] += 1
                    S = sbuild.tile([TP, cfg.MAXI // TP, TP], bf16, tag="S",
                                    name=f"S_{t}_{ci}")
                    seg_col = seg_sb[:, off:off + C]
                    nc.vector.tensor_tensor(
                        S[:, :C, :],
                        seg_col.unsqueeze(2).to_broadcast((TP, C, TP)),
                        iota_sb[:].unsqueeze(1).to_broadcast((TP, C, TP)),
                        mybir.AluOpType.is_equal)
                    for j in range(C):
                        done += 1
                        nc.tensor.matmul(ps[:], S[:, j, :], g[:, j, :],
                                         start=(done == 1),
                                         stop=(done == total))
                return ps

            # ---- pass 1: edge aggregation over h' -------------------------
            # A phase: aggregate half-A edges for every tile, park the
            # partial sums in DRAM (stage1).  Runs as soon as the A
            # AllGather lands, overlapping the B AllGather.
            with nc.named_scope("pass1a"):
                for t in range(NT):
                    ps = seg_gather(t, 0, hfullA_d, hfullB_d, FUSED,
                                    ps_big, "mm")
                    stg = epi.tile([TP, FUSED], f32, tag="stg",
                                   name=f"stg_{t}")
                    nc.vector.tensor_copy(stg[:], ps[:])
                    nc.sync.dma_start(
                        out=stage1_d.ap()[t * TP:(t + 1) * TP, :],
                        in_=stg[:])
            with nc.named_scope("pass1b"):
                for t in range(NT):
                    ps = seg_gather(t, 1, hfullA_d, hfullB_d, FUSED,
                                    ps_big, "mm")
                    stg = epi.tile([TP, FUSED], f32, tag="stg",
                                   name=f"stgi_{t}")
                    nc.sync.dma_start(
                        out=stg[:],
                        in_=stage1_d.ap()[t * TP:(t + 1) * TP, :])
                    dv = dinv_sb[:, t:t + 1]
                    tot_f = epi.tile([TP, FUSED], f32, tag="totf",
                                     name=f"tot_{t}")
                    nc.vector.tensor_tensor(tot_f[:], ps[:], stg[:],
                                            mybir.AluOpType.add)
                    # sig half: relu((psum + selfloop)*dinv + bsig); mask-sum
                    sig_f = epi.tile([TP, HID], f32, tag="sigf")
                    nc.vector.tensor_tensor(sig_f[:], tot_f[:, :HID],
                                            hsh_sb[:, t, :HID],
                                            mybir.AluOpType.add)
                    nc.vector.scalar_tensor_tensor(
                        sig_f[:], sig_f[:], dv, bsig_sb[:],
                        mybir.AluOpType.mult, mybir.AluOpType.add)
                    sig_b = epi.tile([TP, HID], bf16, tag="sigb")
                    nc.scalar.activation(sig_b[:], sig_f[:],
                                         mybir.ActivationFunctionType.Relu)
                    nc.tensor.matmul(s_ps[:], mask_sb[:, t:t + 1], sig_b[:],
                                     start=(t == 0), stop=(t == NT - 1))
                    # conv1 half: (psum + selfloop)*dinv -> bf16
                    c1_f = epi.tile([TP, HID], f32, tag="c1f")
                    nc.vector.tensor_tensor(c1_f[:], tot_f[:, HID:],
                                            hsh_sb[:, t, HID:],
                                            mybir.AluOpType.add)
                    nc.scalar.activation(c1agg_sb[:, t, :], c1_f[:],
                                         mybir.ActivationFunctionType.Copy,
                                         scale=dv)

            # ---- signature allreduce + gamma/beta -------------------------
            with nc.named_scope("signature"):
                s_sb = one.tile([1, HID], f32)
                nc.scalar.copy(s_sb[:], s_ps[:])
                nc.sync.dma_start(out=sin_d.ap(), in_=s_sb[:])
                nc.gpsimd.collective_compute(
                    "AllReduce", mybir.AluOpType.add, replica_groups=rg,
                    ins=[sin_d.ap().opt()], outs=[sout_d.ap().opt()])

                # load s as columns [128, KA]; aug row (=1.0) in col KA-1
                s_col = one.tile([TP, cfg.KA], f32)
                nc.vector.memset(s_col[:], 0.0)
                nc.vector.memset(s_col[0:1, cfg.KA - 1:cfg.KA], 1.0)
                nc.sync.dma_start(
                    out=s_col[:, 0:2],
                    in_=sout_d.ap().rearrange("o (c p) -> (o c) p", p=TP)
                        .transpose([1, 0]))
                s_rep = one.tile([TP, cfg.KA, TP], f32)
                for c in range(cfg.KA):
                    nc.vector.tensor_copy(
                        s_rep[:, c, :],
                        s_col[:, c:c + 1].to_broadcast((TP, TP)))

                gb_sb = {}
                for nm, width in (("wg1", HID), ("wb1", HID),
                                  ("wg2", OUT), ("wb2", OUT)):
                    ps_fc = ps_sm.tile([TP, width], f32, tag="sm", name=nm)
                    for c in range(cfg.KA):
                        nc.tensor.matmul(ps_fc[:], s_rep[:, c, :],
                                         fc_sb[nm][:, c, :],
                                         start=(c == 0), stop=(c == cfg.KA - 1))
                    gb = one.tile([TP, width], f32, name=f"gb_{nm}", tag=nm)
                    nc.scalar.activation(gb[:], ps_fc[:],
                                         mybir.ActivationFunctionType.Tanh)
                    gb_sb[nm] = gb
                # beta + conv bias
                nc.vector.tensor_tensor(gb_sb["wb1"][:], gb_sb["wb1"][:],
                                        b1c_sb[:], mybir.AluOpType.add)
                nc.vector.tensor_tensor(gb_sb["wb2"][:], gb_sb["wb2"][:],
                                        b2c_sb[:], mybir.AluOpType.add)

            # ---- encoder local: FiLM + relu + LN + conv2 matmul -----------
            def layernorm(dst_ap, src_ap, width):
                st6 = small.tile([TP, 6], f32, tag="st6", name="st6")
                mv = small.tile([TP, 2], f32, tag="mv", name="mv")
                nc.vector.bn_stats(st6[:], src_ap)
                nc.vector.bn_aggr(mv[:], st6[:])
                std = small.tile([TP, 1], f32, tag="std", name="std")
                nc.scalar.activation(std[:], mv[:, 1:2],
                                     mybir.ActivationFunctionType.Sqrt,
                                     bias=eps_sb[:, 0:1])
                rstd = small.tile([TP, 1], f32, tag="rstd", name="rstd")
                nc.vector.reciprocal(rstd[:], std[:])
                nmr = small.tile([TP, 1], f32, tag="nmr", name="nmr")
                nc.vector.scalar_tensor_tensor(
                    nmr[:], mv[:, 0:1], -1.0, rstd[:],
                    mybir.AluOpType.mult, mybir.AluOpType.mult)
                nc.scalar.activation(dst_ap, src_ap,
                                     mybir.ActivationFunctionType.Identity,
                                     bias=nmr[:, 0:1], scale=rstd[:, 0:1])

            with nc.named_scope("encoder_local"):
                for t in range(NT):
                    h_f = epi.tile([TP, HID], f32, tag="hf", name=f"h_{t}")
                    nc.vector.tensor_tensor(h_f[:], c1agg_sb[:, t, :],
                                            gb_sb["wg1"][:],
                                            mybir.AluOpType.mult)
                    nc.vector.tensor_tensor(h_f[:], h_f[:], gb_sb["wb1"][:],
                                            mybir.AluOpType.add)
                    nc.scalar.activation(h_f[:], h_f[:],
                                         mybir.ActivationFunctionType.Relu)
                    h1 = epi.tile([TP, HID], bf16, tag="h1", name=f"h1_{t}")
                    layernorm(h1[:], h_f[:], HID)
                    # transpose h1 tile and matmul with w2
                    h1T = epi.tile([TP, cfg.KH, TP], bf16, tag="h1T",
                                   name=f"h1T_{t}")
                    for c in range(cfg.KH):
                        ps_t = ps_sm.tile([TP, TP], bf16, tag="sm",
                                          name=f"tr_{t}_{c}")
                        nc.tensor.transpose(ps_t[:],
                                            h1[:, c * TP:(c + 1) * TP],
                                            ident_sb[:])
                        nc.vector.tensor_copy(h1T[:, c, :], ps_t[:])
                    ps2 = ps_sm.tile([TP, OUT], f32, tag="sm", name=f"w2_{t}")
                    for c in range(cfg.KH):
                        nc.tensor.matmul(ps2[:], h1T[:, c, :], w2_sb[:, c, :],
                                         start=(c == 0), stop=(c == cfg.KH - 1))
                    nc.scalar.activation(tp_sb[:, t, :], ps2[:],
                                         mybir.ActivationFunctionType.Copy,
                                         scale=dinv_sb[:, t:t + 1])
                    if t < NT_A:
                        dst_ap = tshA_d.ap()[t * TP:(t + 1) * TP, :]
                    else:
                        dst_ap = tshB_d.ap()[(t - NT_A) * TP:
                                             (t - NT_A + 1) * TP, :]
                    nc.sync.dma_start(out=dst_ap, in_=tp_sb[:, t, :])
                    if t == NT_A - 1:
                        nc.gpsimd.collective_compute(
                            "AllGather", mybir.AluOpType.bypass,
                            replica_groups=rg,
                            ins=[tshA_d.ap().opt()],
                            outs=[tfullA_d.ap().opt()])
                nc.gpsimd.collective_compute(
                    "AllGather", mybir.AluOpType.bypass, replica_groups=rg,
                    ins=[tshB_d.ap().opt()], outs=[tfullB_d.ap().opt()])

            # ---- pass 2: edge aggregation over t' -------------------------
            with nc.named_scope("pass2a"):
                for t in range(NT):
                    ps = seg_gather(t, 0, tfullA_d, tfullB_d, OUT,
                                    ps_big, "mm2")
                    stg = epi.tile([TP, OUT], f32, tag="stg2",
                                   name=f"st2_{t}")
                    nc.vector.tensor_copy(stg[:], ps[:])
                    nc.sync.dma_start(
                        out=stage2_d.ap()[t * TP:(t + 1) * TP, :],
                        in_=stg[:])
            with nc.named_scope("pass2b"):
                for t in range(NT):
                    ps = seg_gather(t, 1, tfullA_d, tfullB_d, OUT,
                                    ps_big, "mm2")
                    stg = epi.tile([TP, OUT], f32, tag="stg2",
                                   name=f"st2i_{t}")
                    nc.sync.dma_start(
                        out=stg[:],
                        in_=stage2_d.ap()[t * TP:(t + 1) * TP, :])
                    dv = dinv_sb[:, t:t + 1]
                    o_f = epi.tile([TP, OUT], f32, tag="of", name=f"o_{t}")
                    nc.vector.tensor_tensor(o_f[:], ps[:], stg[:],
                                            mybir.AluOpType.add)
                    nc.vector.tensor_tensor(o_f[:], o_f[:],
                                            tp_sb[:, t, :],
                                            mybir.AluOpType.add)
                    # gamma2 * (dinv * agg) + (beta2 + b2)
                    nc.vector.scalar_tensor_tensor(
                        o_f[:], o_f[:], dv, gb_sb["wg2"][:],
                        mybir.AluOpType.mult, mybir.AluOpType.mult)
                    nc.vector.tensor_tensor(o_f[:], o_f[:], gb_sb["wb2"][:],
                                            mybir.AluOpType.add)
                    o_ln = epi.tile([TP, OUT], f32, tag="oln", name=f"ol_{t}")
                    layernorm(o_ln[:], o_f[:], OUT)
                    nc.sync.dma_start(out=out_d.ap()[t * TP:(t + 1) * TP, :],
                                      in_=o_ln[:])

    nc.compile()
    return nc

# ---------------------------------------------------------------- runner ----


_CACHE = {}


def _get_program(cfg, calls):
    key = (cfg.NT, calls)
    if key not in _CACHE:
        _CACHE[key] = build_program(cfg, calls)
    return _CACHE[key]


def run(inputs, cfg=FULL, trace=False, **kw):
    in_maps, calls = make_in_maps(inputs, cfg)
    nc = _get_program(cfg, calls)
    res = bass_utils.run_bass_kernel_spmd(
        nc, in_maps, core_ids=list(range(cfg.NC)), trace=trace, **kw)
    out = np.concatenate([res.results[c]["out"] for c in range(cfg.NC)],
                         axis=0)[: cfg.n_real]
    return out.astype(np.float32), res


def kernel(**inputs):
    out, _ = run(inputs, FULL)
    return out


# revision 13
# speedup vs baseline: 1.6511x; 1.1176x over previous
"""Trainium2 Bass kernel for nn_MetaSignatureEncoder (GCN encoder with FiLM
signature conditioning), distributed over 8 NeuronCores.

Strategy (graph/data parallel, per the sharding hint):
  - Nodes are padded to NPAD = 8*49*128 = 50176 and sharded contiguously
    across the 8 cores (6272 nodes/core, 49 dst tiles of 128).
  - norm[e] = dinv[src]*dinv[dst] factors out of the message sum, so rows are
    pre-scaled once (h' = dinv * (x @ W)), edges aggregate UNWEIGHTED, and the
    dst factor is applied after aggregation.  Self-loops are a local add.
  - Phase 1: each core computes h' for its shard (PE matmul), AllGather
    replicates the full [NPAD, 512] bf16 table to every core's HBM.
  - Pass 1 (edges): per dst-tile of 128 nodes, dma_gather fetches the h' rows
    of all in-edges (grouped per tile on the host, int16 indices, node space
    split in two segments because gather indices are int16), and the
    segment-sum is done on the TensorEngine: for each chunk of 128 messages a
    one-hot S matrix ([128,128], S[j,d] = 1 iff dst(msg j) == d, built by one
    DVE is_equal against an iota row) is matmul'd with the gathered rows,
    accumulating in PSUM.  The number of 128-message chunks per (tile, seg)
    is baked into the program per problem instance (no padded descriptors).
  - The graph signature s = sum_n relu(agg_sig + b) is reduced over nodes
    with a mask-vector matmul (PE) and AllReduce'd (f32: the fc preacts are
    ~1e4 scale, tanh sign flips forbid bf16 here).
  - gamma/beta FiLM vectors are computed redundantly on every core with f32
    matmuls whose lhsT is s broadcast along the free axis.
  - Encoder: FiLM + relu + LN (bn_stats/bn_aggr) per tile, conv2 matmul via
    PE transpose, dinv scale, AllGather of the [NPAD, 128] bf16 table,
    second edge pass identical in structure, FiLM + LN epilogue, output.

kernel(**inputs) takes the FULL problem inputs and returns the FULL output.
"""
import os
import sys
import math
import numpy as np
import ml_dtypes

sys.path.insert(0, "/opt/trn_rl_repo")

from concourse import bass, bacc, tile, mybir
from concourse import bass_utils

BF16 = ml_dtypes.bfloat16
dt = mybir.dt

# ---------------------------------------------------------------- config ----


class Cfg:
    def __init__(self, NT=49, n_real=50000, n_edges=800000):
        self.NC = 8           # cores
        self.TP = 128         # partitions / dst-tile size
        self.NT = NT          # dst tiles per core
        self.SPLITS = 2       # src-space segments (int16 idx < 32768)
        self.MAXI = 1024      # max dma_gather descriptors per call
        self.IN_CH = 256
        self.HID = 256
        self.OUT = 128
        self.FUSED = self.HID + self.HID   # sig(256) | conv1(256)
        self.KX = self.IN_CH // 128        # K chunks for x matmul
        self.KH = self.HID // 128          # K chunks for conv2 matmul
        self.KA = 3                        # K chunks for augmented fc matmuls
        self.SHARD = self.NT * self.TP
        self.NPAD = self.NC * self.SHARD
        self.NT_A = 17                     # tiles in AllGather half A (17..32: both halves fit int16)
        self.ROWS_A = self.NT_A * self.TP  # 3200 local rows in half A
        # half-table sizes (all cores' A rows; all cores' B rows)
        self.TBL_A = self.NC * self.ROWS_A
        self.TBL_B = self.NPAD - self.TBL_A
        self.n_real = n_real
        self.n_edges = n_edges
        self.LN_EPS = 1e-5


FULL = Cfg()

# ------------------------------------------------------------ host side -----


def _wrap16(vals, nrows=128):
    """dma_gather index layout: idx j at [j % 16, j // 16], replicated to all
    8 q7 core groups (rows 16k+p == row p)."""
    n = vals.shape[0]
    assert n % 16 == 0
    w = vals.reshape(n // 16, 16).T          # [16, n/16]
    return np.tile(w, (nrows // 16, 1))      # [128, n/16]


def _pmaj(vals, TP=128):
    """[NT*TP] -> [TP, NT] partition-major (tile t col, partition p row)."""
    return np.ascontiguousarray(vals.reshape(-1, TP).T)


def preprocess(edge_index, cfg):
    """Integer-only graph preprocessing -> per-core gather structures.

    Per core: a flat list of dma_gather calls, each for one (dst-tile, seg)
    with a compile-time chunk count C_i = ceil(n_i / 128) <= MAXI/128.
    Returns (deg, per_core, calls) where calls[i] = (tile, C_i) is shared
    by all cores (max over cores, so one SPMD program fits all).
    """
    src = np.asarray(edge_index[0], dtype=np.int64)
    dst = np.asarray(edge_index[1], dtype=np.int64)
    deg = np.bincount(src, minlength=cfg.NPAD).astype(np.float32) + 1.0

    SP = cfg.SPLITS
    # Segment = which half of its owner shard a source row lives in.
    # Half A = local rows [0, ROWS_A), half B = [ROWS_A, SHARD).  The
    # AllGather runs as two collectives (A then B), so gathers against
    # table A can start while B is still in flight.
    RA, RB = cfg.ROWS_A, cfg.SHARD - cfg.ROWS_A
    s_core = src // cfg.SHARD
    s_lr = src % cfg.SHARD
    s_seg = (s_lr >= RA).astype(np.int64)
    s_rel = np.where(s_seg == 0, s_core * RA + s_lr,
                     s_core * RB + (s_lr - RA))

    # group per (core, tile, seg)
    per_core_lists = []
    shard_of = dst // cfg.SHARD
    for c in range(cfg.NC):
        m = shard_of == c
        s_rel_c, s_seg_c = s_rel[m], s_seg[m]
        d_c = dst[m] - c * cfg.SHARD
        tile_of = d_c // cfg.TP
        order = np.argsort(tile_of, kind="stable")
        s_rel_c, s_seg_c = s_rel_c[order], s_seg_c[order]
        d_c, tile_of = d_c[order], tile_of[order]
        bounds = np.searchsorted(tile_of, np.arange(cfg.NT + 1))
        tiles = []
        for t in range(cfg.NT):
            sl = slice(bounds[t], bounds[t + 1])
            s_t, g_t = s_rel_c[sl], s_seg_c[sl]
            d_t = d_c[sl] - t * cfg.TP
            segs = []
            for k in range(SP):
                a = g_t == k
                o = np.argsort(s_t[a], kind="stable")
                segs.append((s_t[a][o], d_t[a][o]))
            tiles.append(segs)
        per_core_lists.append(tiles)

    # uniform call structure: per (tile, seg) C = ceil(max_c n / 128),
    # split into multiple calls if C > MAXI/128
    maxc = cfg.MAXI // cfg.TP
    calls = []      # list of (tile, seg, chunk_count)
    for t in range(cfg.NT):
        for k in range(SP):
            nmax = max(len(per_core_lists[c][t][k][0])
                       for c in range(cfg.NC))
            C = max(1, math.ceil(nmax / cfg.TP))
            while C > maxc:
                calls.append((t, k, maxc))
                C -= maxc
            calls.append((t, k, C))

    per_core = []
    for c in range(cfg.NC):
        idx_parts, seg_parts = [], []
        consumed = {}
        for (t, k, C) in calls:
            n_slots = C * cfg.TP
            s_all, d_all = per_core_lists[c][t][k]
            lo = consumed.get((t, k), 0)
            hi = min(lo + n_slots, len(s_all))
            consumed[(t, k)] = hi
            ii = np.zeros(n_slots, np.int64)
            ss = -np.ones(n_slots, np.float32)
            ii[: hi - lo] = s_all[lo:hi]
            ss[: hi - lo] = d_all[lo:hi]
            idx_parts.append(_wrap16(ii).astype(np.int16))
            seg_parts.append(np.ascontiguousarray(
                ss.reshape(C, cfg.TP).T).astype(BF16))
        per_core.append({
            "idx": np.concatenate(idx_parts, axis=1),
            "seg": np.concatenate(seg_parts, axis=1),
        })
    return deg, per_core, tuple(calls)


def make_in_maps(inputs, cfg):
    """Build the per-core input maps for run_bass_kernel_spmd."""
    x = np.asarray(inputs["x"], np.float32)
    deg, per_core, calls = preprocess(np.asarray(inputs["edge_index"]), cfg)

    xp = np.zeros((cfg.NPAD, cfg.IN_CH), np.float32)
    xp[: x.shape[0]] = x

    def chunks(a, k):  # [K*128, N] -> [K, 128, N]
        return np.ascontiguousarray(a.reshape(k, 128, a.shape[1]))

    wf = np.concatenate([np.asarray(inputs["sig_conv_w"], np.float32),
                         np.asarray(inputs["conv1_w"], np.float32)], axis=1)

    def aug(w, b):  # [N, K] weight + [N] bias -> [KA, 128, N] f32 (w.T | b | 0)
        wt = np.asarray(w, np.float32).T
        a = np.zeros((cfg.KA * 128, wt.shape[1]), np.float32)
        a[: wt.shape[0]] = wt
        a[wt.shape[0]] = np.asarray(b, np.float32)
        return chunks(a, cfg.KA)

    shared = {
        "wf": chunks(wf, cfg.KX).astype(BF16),
        "w2": chunks(np.asarray(inputs["conv2_w"], np.float32),
                     cfg.KH).astype(BF16),
        "wg1": aug(inputs["fc1_w"], inputs["fc1_b"]),
        "wb1": aug(inputs["fc2_w"], inputs["fc2_b"]),
        "wg2": aug(inputs["fc3_w"], inputs["fc3_b"]),
        "wb2": aug(inputs["fc4_w"], inputs["fc4_b"]),
        "bsig": np.broadcast_to(np.asarray(inputs["sig_conv_b"], np.float32),
                                (128, cfg.HID)).copy(),
        "b1c": np.broadcast_to(np.asarray(inputs["conv1_b"], np.float32),
                               (128, cfg.HID)).copy(),
        "b2c": np.broadcast_to(np.asarray(inputs["conv2_b"], np.float32),
                               (128, cfg.OUT)).copy(),
        "iota": np.broadcast_to(np.arange(128, dtype=np.float32),
                                (128, 128)).astype(BF16).copy(),
        "ident": np.eye(128, dtype=np.float32).astype(BF16),
    }

    in_maps = []
    node_ids = np.arange(cfg.SHARD)
    for c in range(cfg.NC):
        sl = slice(c * cfg.SHARD, (c + 1) * cfg.SHARD)
        gids = node_ids + c * cfg.SHARD
        m = dict(shared)
        m["xT"] = chunks(np.ascontiguousarray(xp[sl].T), cfg.KX).astype(BF16)
        m["deg"] = _pmaj(deg[sl]).copy()
        m["sigmask"] = _pmaj((gids < cfg.n_real).astype(np.float32)).astype(BF16)
        m.update(per_core[c])
        in_maps.append(m)
    return in_maps, calls

# --------------------------------------------------------------- builder ----


def build_program(cfg, calls):
    nc = bacc.Bacc("TRN2", target_bir_lowering=False, debug=False,
                   num_devices=cfg.NC,
                   num_swdge_queues=4)
    f32, bf16, i16 = dt.float32, dt.bfloat16, dt.int16
    TP, NT = cfg.TP, cfg.NT
    HID, OUT, FUSED = cfg.HID, cfg.OUT, cfg.FUSED
    SP = cfg.SPLITS
    TOTC = sum(C for (_, _, C) in calls)

    def inp(name, shape, dtype):
        return nc.dram_tensor(name, shape, dtype, kind="ExternalInput")

    xT_d = inp("xT", [cfg.KX, TP, cfg.SHARD], bf16)
    wf_d = inp("wf", [cfg.KX, TP, FUSED], bf16)
    w2_d = inp("w2", [cfg.KH, TP, OUT], bf16)
    wg1_d = inp("wg1", [cfg.KA, TP, HID], f32)
    wb1_d = inp("wb1", [cfg.KA, TP, HID], f32)
    wg2_d = inp("wg2", [cfg.KA, TP, OUT], f32)
    wb2_d = inp("wb2", [cfg.KA, TP, OUT], f32)
    bsig_d = inp("bsig", [TP, HID], f32)
    b1c_d = inp("b1c", [TP, HID], f32)
    b2c_d = inp("b2c", [TP, OUT], f32)
    iota_d = inp("iota", [TP, TP], bf16)
    ident_d = inp("ident", [TP, TP], bf16)
    deg_d = inp("deg", [TP, NT], f32)
    mask_d = inp("sigmask", [TP, NT], bf16)
    idx_d = inp("idx", [TP, TOTC * 8], i16)
    seg_d = inp("seg", [TP, TOTC], bf16)

    out_d = nc.dram_tensor("out", [cfg.SHARD, OUT], f32, kind="ExternalOutput")

    # internal DRAM (collective bounce buffers, split in two halves so the
    # second AllGather can overlap gathers against the first half)
    RA, RB = cfg.ROWS_A, cfg.SHARD - cfg.ROWS_A
    hshA_d = nc.dram_tensor("hshA", [RA, FUSED], bf16)
    hshB_d = nc.dram_tensor("hshB", [RB, FUSED], bf16)
    hfullA_d = nc.dram_tensor("hfullA", [cfg.TBL_A, FUSED], bf16,
                              addr_space="Shared")
    hfullB_d = nc.dram_tensor("hfullB", [cfg.TBL_B, FUSED], bf16,
                              addr_space="Shared")
    tshA_d = nc.dram_tensor("tshA", [RA, OUT], bf16)
    tshB_d = nc.dram_tensor("tshB", [RB, OUT], bf16)
    tfullA_d = nc.dram_tensor("tfullA", [cfg.TBL_A, OUT], bf16,
                              addr_space="Shared")
    tfullB_d = nc.dram_tensor("tfullB", [cfg.TBL_B, OUT], bf16,
                              addr_space="Shared")
    stage1_d = nc.dram_tensor("stage1", [cfg.SHARD, FUSED], f32)
    stage2_d = nc.dram_tensor("stage2", [cfg.SHARD, OUT], f32)
    sin_d = nc.dram_tensor("sin", [1, HID], f32)
    sout_d = nc.dram_tensor("sout", [1, HID], f32, addr_space="Shared")

    rg = [list(range(cfg.NC))]

    # per-call cumulative offsets into idx/seg
    call_off = []
    o = 0
    for (_, _, C) in calls:
        call_off.append(o)
        o += C

    # group calls per tile (they are consecutive by construction)
    tile_calls = {t: [] for t in range(NT)}
    for i, (t, k, C) in enumerate(calls):
        tile_calls[t].append((i, k, C))

    with tile.TileContext(nc) as tc:
        with (
            tc.tile_pool(name="const", bufs=1) as const,
            tc.tile_pool(name="persist", bufs=1) as persist,
            tc.tile_pool(name="xload", bufs=3) as xload,
            tc.tile_pool(name="gat", bufs=4) as gat,
            tc.tile_pool(name="sbuild", bufs=4) as sbuild,
            tc.tile_pool(name="epi", bufs=3) as epi,
            tc.tile_pool(name="small", bufs=4) as small,
            tc.tile_pool(name="one", bufs=1) as one,
            tc.tile_pool(name="ps_big", bufs=2, space="PSUM") as ps_big,
            tc.tile_pool(name="ps_sig", bufs=1, space="PSUM") as ps_sig,
            tc.tile_pool(name="ps_sm", bufs=3, space="PSUM") as ps_sm,
        ):
            # ---- load constants -------------------------------------------
            wf_sb = const.tile([TP, cfg.KX, FUSED], bf16)
            w2_sb = const.tile([TP, cfg.KH, OUT], bf16)
            nc.sync.dma_start(out=wf_sb[:], in_=wf_d.ap().transpose([1, 0, 2]))
            nc.sync.dma_start(out=w2_sb[:], in_=w2_d.ap().transpose([1, 0, 2]))
            fc_sb = {}
            for nm, d, width in (("wg1", wg1_d, HID), ("wb1", wb1_d, HID),
                                 ("wg2", wg2_d, OUT), ("wb2", wb2_d, OUT)):
                t_ = const.tile([TP, cfg.KA, width], f32, name=nm)
                nc.sync.dma_start(out=t_[:], in_=d.ap().transpose([1, 0, 2]))
                fc_sb[nm] = t_
            bsig_sb = const.tile([TP, HID], f32)
            b1c_sb = const.tile([TP, HID], f32)
            b2c_sb = const.tile([TP, OUT], f32)
            iota_sb = const.tile([TP, TP], bf16)
            ident_sb = const.tile([TP, TP], bf16)
            deg_sb = const.tile([TP, NT], f32)
            mask_sb = const.tile([TP, NT], bf16)
            idx_sb = const.tile([TP, TOTC * 8], i16)
            seg_sb = const.tile([TP, TOTC], bf16)
            for t_, d in ((bsig_sb, bsig_d), (b1c_sb, b1c_d), (b2c_sb, b2c_d),
                          (iota_sb, iota_d), (ident_sb, ident_d),
                          (deg_sb, deg_d), (mask_sb, mask_d),
                          (idx_sb, idx_d), (seg_sb, seg_d)):
                nc.sync.dma_start(out=t_[:], in_=d.ap())

            eps_sb = const.tile([TP, 1], f32)
            nc.vector.memset(eps_sb[:], cfg.LN_EPS)

            # dinv = 1/sqrt(deg)
            dinv_sb = const.tile([TP, NT], f32)
            nc.scalar.sqrt(dinv_sb[:], deg_sb[:])
            nc.vector.reciprocal(dinv_sb[:], dinv_sb[:])

            # persistent per-shard state
            hsh_sb = persist.tile([TP, NT, FUSED], bf16)    # h' shard
            c1agg_sb = persist.tile([TP, NT, HID], bf16)    # conv1 aggregate
            tp_sb = persist.tile([TP, NT, OUT], bf16)       # t' shard

            # ---- phase 1: h' = dinv * (x @ [Wsig|W1]), allgather ----------
            NT_A = cfg.NT_A
            with nc.named_scope("phase1"):
                for t in range(NT):
                    xt = xload.tile([TP, cfg.KX, TP], bf16, tag="xt")
                    for k in range(cfg.KX):
                        nc.sync.dma_start(
                            out=xt[:, k, :],
                            in_=xT_d.ap()[k, :, t * TP:(t + 1) * TP])
                    ps = ps_big.tile([TP, FUSED], f32, tag="mm")
                    for k in range(cfg.KX):
                        nc.tensor.matmul(ps[:], xt[:, k, :], wf_sb[:, k, :],
                                         start=(k == 0), stop=(k == cfg.KX - 1))
                    nc.scalar.activation(hsh_sb[:, t, :], ps[:],
                                         mybir.ActivationFunctionType.Copy,
                                         scale=dinv_sb[:, t:t + 1])
                    if t < NT_A:
                        dst_ap = hshA_d.ap()[t * TP:(t + 1) * TP, :]
                    else:
                        dst_ap = hshB_d.ap()[(t - NT_A) * TP:
                                             (t - NT_A + 1) * TP, :]
                    nc.sync.dma_start(out=dst_ap, in_=hsh_sb[:, t, :])
                    if t == NT_A - 1:
                        nc.gpsimd.collective_compute(
                            "AllGather", mybir.AluOpType.bypass,
                            replica_groups=rg,
                            ins=[hshA_d.ap().opt()],
                            outs=[hfullA_d.ap().opt()])
                nc.gpsimd.collective_compute(
                    "AllGather", mybir.AluOpType.bypass, replica_groups=rg,
                    ins=[hshB_d.ap().opt()], outs=[hfullB_d.ap().opt()])

            # ---- edge aggregation machinery -------------------------------
            s_ps = ps_sig.tile([1, HID], f32)
            qctr = [0]

            def seg_gather(t, k, tblA, tblB, width, ps_pool, tag):
                """gather + segsum-matmul over segment k of dst-tile t."""
                ps = ps_pool.tile([TP, width], f32, tag=tag,
                                  name=f"ps{tag}_{t}_{k}")
                ncalls = [c for c in tile_calls[t] if c[1] == k]
                total = sum(C for (_, _, C) in ncalls)
                done = 0
                for (ci, kk, C) in ncalls:
                    table = (tblA if kk == 0 else tblB).ap()
                    off = call_off[ci]
                    g = gat.tile([TP, cfg.MAXI // TP, width],
                                 bf16, tag=f"g{width}", name=f"g_{t}_{ci}")
                    nc.gpsimd.dma_gather(
                        out_ap=g[:, :C, :],
                        in_ap=table,
                        idxs_ap=idx_sb[:, off * 8:(off + C) * 8],
                        num_idxs=C * TP,
                        num_idxs_reg=C * TP,
                        elem_size=width,
                        queue_num=qctr[0] % 4,
                    )
                    qctr[0] += 1
                    S = sbuild.tile([TP, cfg.MAXI // TP, TP], bf16, tag="S",
                                    name=f"S_{t}_{ci}")
                    seg_col = seg_sb[:, off:off + C]
                    nc.vector.tensor_tensor(
                        S[:, :C, :],
                        seg_col.unsqueeze(2).to_broadcast((TP, C, TP)),
                        iota_sb[:].unsqueeze(1).to_broadcast((TP, C, TP)),
                        mybir.AluOpType.is_equal)
                    for j in range(C):
                        done += 1
                        nc.tensor.matmul(ps[:], S[:, j, :], g[:, j, :],
                                         start=(done == 1),
                                         stop=(done == total))
                return ps

            # ---- pass 1: edge aggregation over h' -------------------------
            # A phase: aggregate half-A edges for every tile, park the
            # partial sums in DRAM (stage1).  Runs as soon as the A
            # AllGather lands, overlapping the B AllGather.
            with nc.named_scope("pass1a"):
                for t in range(NT):
                    ps = seg_gather(t, 0, hfullA_d, hfullB_d, FUSED,
                                    ps_big, "mm")
                    stg = epi.tile([TP, FUSED], f32, tag="stg",
                                   name=f"stg_{t}")
                    nc.vector.tensor_copy(stg[:], ps[:])
                    nc.sync.dma_start(
                        out=stage1_d.ap()[t * TP:(t + 1) * TP, :],
                        in_=stg[:])
            with nc.named_scope("pass1b"):
                for t in range(NT):
                    ps = seg_gather(t, 1, hfullA_d, hfullB_d, FUSED,
                                    ps_big, "mm")
                    stg = epi.tile([TP, FUSED], f32, tag="stg",
                                   name=f"stgi_{t}")
                    nc.sync.dma_start(
                        out=stg[:],
                        in_=stage1_d.ap()[t * TP:(t + 1) * TP, :])
                    dv = dinv_sb[:, t:t + 1]
                    tot_f = epi.tile([TP, FUSED], f32, tag="totf",
                                     name=f"tot_{t}")
                    nc.vector.tensor_tensor(tot_f[:], ps[:], stg[:],
                                            mybir.AluOpType.add)
                    # sig half: relu((psum + selfloop)*dinv + bsig); mask-sum
                    sig_f = epi.tile([TP, HID], f32, tag="sigf")
                    nc.vector.tensor_tensor(sig_f[:], tot_f[:, :HID],
                                            hsh_sb[:, t, :HID],
                                            mybir.AluOpType.add)
                    nc.vector.scalar_tensor_tensor(
                        sig_f[:], sig_f[:], dv, bsig_sb[:],
                        mybir.AluOpType.mult, mybir.AluOpType.add)
                    sig_b = epi.tile([TP, HID], bf16, tag="sigb")
                    nc.scalar.activation(sig_b[:], sig_f[:],
                                         mybir.ActivationFunctionType.Relu)
                    nc.tensor.matmul(s_ps[:], mask_sb[:, t:t + 1], sig_b[:],
                                     start=(t == 0), stop=(t == NT - 1))
                    # conv1 half: (psum + selfloop)*dinv -> bf16
                    c1_f = epi.tile([TP, HID], f32, tag="c1f")
                    nc.vector.tensor_tensor(c1_f[:], tot_f[:, HID:],
                                            hsh_sb[:, t, HID:],
                                            mybir.AluOpType.add)
                    nc.scalar.activation(c1agg_sb[:, t, :], c1_f[:],
                                         mybir.ActivationFunctionType.Copy,
                                         scale=dv)

            # ---- signature allreduce + gamma/beta -------------------------
            with nc.named_scope("signature"):
                s_sb = one.tile([1, HID], f32)
                nc.scalar.copy(s_sb[:], s_ps[:])
                nc.sync.dma_start(out=sin_d.ap(), in_=s_sb[:])
                nc.gpsimd.collective_compute(
                    "AllReduce", mybir.AluOpType.add, replica_groups=rg,
                    ins=[sin_d.ap().opt()], outs=[sout_d.ap().opt()])

                # load s as columns [128, KA]; aug row (=1.0) in col KA-1
                s_col = one.tile([TP, cfg.KA], f32)
                nc.vector.memset(s_col[:], 0.0)
                nc.vector.memset(s_col[0:1, cfg.KA - 1:cfg.KA], 1.0)
                nc.sync.dma_start(
                    out=s_col[:, 0:2],
                    in_=sout_d.ap().rearrange("o (c p) -> (o c) p", p=TP)
                        .transpose([1, 0]))
                s_rep = one.tile([TP, cfg.KA, TP], f32)
                for c in range(cfg.KA):
                    nc.vector.tensor_copy(
                        s_rep[:, c, :],
                        s_col[:, c:c + 1].to_broadcast((TP, TP)))

                gb_sb = {}
                for nm, width in (("wg1", HID), ("wb1", HID),
                                  ("wg2", OUT), ("wb2", OUT)):
                    ps_fc = ps_sm.tile([TP, width], f32, tag="sm", name=nm)
                    for c in range(cfg.KA):
                        nc.tensor.matmul(ps_fc[:], s_rep[:, c, :],
                                         fc_sb[nm][:, c, :],
                                         start=(c == 0), stop=(c == cfg.KA - 1))
                    gb = one.tile([TP, width], f32, name=f"gb_{nm}", tag=nm)
                    nc.scalar.activation(gb[:], ps_fc[:],
                                         mybir.ActivationFunctionType.Tanh)
                    gb_sb[nm] = gb
                # beta + conv bias
                nc.vector.tensor_tensor(gb_sb["wb1"][:], gb_sb["wb1"][:],
                                        b1c_sb[:], mybir.AluOpType.add)
                nc.vector.tensor_tensor(gb_sb["wb2"][:], gb_sb["wb2"][:],
                                        b2c_sb[:], mybir.AluOpType.add)

            # ---- encoder local: FiLM + relu + LN + conv2 matmul -----------
            def layernorm(dst_ap, src_ap, width):
                st6 = small.tile([TP, 6], f32, tag="st6", name="st6")
                mv = small.tile([TP, 2], f32, tag="mv", name="mv")
                nc.vector.bn_stats(st6[:], src_ap)
                nc.vector.bn_aggr(mv[:], st6[:])
                std = small.tile([TP, 1], f32, tag="std", name="std")
                nc.scalar.activation(std[:], mv[:, 1:2],
                                     mybir.ActivationFunctionType.Sqrt,
                                     bias=eps_sb[:, 0:1])
                rstd = small.tile([TP, 1], f32, tag="rstd", name="rstd")
                nc.vector.reciprocal(rstd[:], std[:])
                nmr = small.tile([TP, 1], f32, tag="nmr", name="nmr")
                nc.vector.scalar_tensor_tensor(
                    nmr[:], mv[:, 0:1], -1.0, rstd[:],
                    mybir.AluOpType.mult, mybir.AluOpType.mult)
                nc.scalar.activation(dst_ap, src_ap,
                                     mybir.ActivationFunctionType.Identity,
                                     bias=nmr[:, 0:1], scale=rstd[:, 0:1])

            with nc.named_scope("encoder_local"):
                for t in range(NT):
                    h_f = epi.tile([TP, HID], f32, tag="hf", name=f"h_{t}")
                    nc.vector.tensor_tensor(h_f[:], c1agg_sb[:, t, :],
                                            gb_sb["wg1"][:],
                                            mybir.AluOpType.mult)
                    nc.vector.tensor_tensor(h_f[:], h_f[:], gb_sb["wb1"][:],
                                            mybir.AluOpType.add)
                    nc.scalar.activation(h_f[:], h_f[:],
                                         mybir.ActivationFunctionType.Relu)
                    h1 = epi.tile([TP, HID], bf16, tag="h1", name=f"h1_{t}")
                    layernorm(h1[:], h_f[:], HID)
                    # transpose h1 tile and matmul with w2
                    h1T = epi.tile([TP, cfg.KH, TP], bf16, tag="h1T",
                                   name=f"h1T_{t}")
                    for c in range(cfg.KH):
                        ps_t = ps_sm.tile([TP, TP], bf16, tag="sm",
                                          name=f"tr_{t}_{c}")
                        nc.tensor.transpose(ps_t[:],
                                            h1[:, c * TP:(c + 1) * TP],
                                            ident_sb[:])
                        nc.vector.tensor_copy(h1T[:, c, :], ps_t[:])
                    ps2 = ps_sm.tile([TP, OUT], f32, tag="sm", name=f"w2_{t}")
                    for c in range(cfg.KH):
                        nc.tensor.matmul(ps2[:], h1T[:, c, :], w2_sb[:, c, :],
                                         start=(c == 0), stop=(c == cfg.KH - 1))
                    nc.scalar.activation(tp_sb[:, t, :], ps2[:],
                                         mybir.ActivationFunctionType.Copy,
                                         scale=dinv_sb[:, t:t + 1])
                    if t < NT_A:
                        dst_ap = tshA_d.ap()[t * TP:(t + 1) * TP, :]
                    else:
                        dst_ap = tshB_d.ap()[(t - NT_A) * TP:
                                             (t - NT_A + 1) * TP, :]
                    nc.sync.dma_start(out=dst_ap, in_=tp_sb[:, t, :])
                    if t == NT_A - 1:
                        nc.gpsimd.collective_compute(
                            "AllGather", mybir.AluOpType.bypass,
                            replica_groups=rg,
                            ins=[tshA_d.ap().opt()],
                            outs=[tfullA_d.ap().opt()])
                nc.gpsimd.collective_compute(
                    "AllGather", mybir.AluOpType.bypass, replica_groups=rg,
                    ins=[tshB_d.ap().opt()], outs=[tfullB_d.ap().opt()])

            # ---- pass 2: edge aggregation over t' -------------------------
            with nc.named_scope("pass2a"):
                for t in range(NT):
                    ps = seg_gather(t, 0, tfullA_d, tfullB_d, OUT,
                                    ps_big, "mm2")
                    stg = epi.tile([TP, OUT], f32, tag="stg2",
                                   name=f"st2_{t}")
                    nc.vector.tensor_copy(stg[:], ps[:])
                    nc.sync.dma_start(
                        out=stage2_d.ap()[t * TP:(t + 1) * TP, :],
                        in_=stg[:])
            with nc.named_scope("pass2b"):
                for t in range(NT):
                    ps = seg_gather(t, 1, tfullA_d, tfullB_d, OUT,
                                    ps_big, "mm2")
                    stg = epi.tile([TP, OUT], f32, tag="stg2",
                                   name=f"st2i_{t}")
                    nc.sync.dma_start(
                        out=stg[:],
                        in_=stage2_d.ap()[t * TP:(t + 1) * TP, :])
                    dv = dinv_sb[:, t:t + 1]
                    o_f = epi.tile([TP, OUT], f32, tag="of", name=f"o_{t}")
                    nc.vector.tensor_tensor(o_f[:], ps[:], stg[:],
                                            mybir.AluOpType.add)
                    nc.vector.tensor_tensor(o_f[:], o_f[:],
                                            tp_sb[:, t, :],
                                            mybir.AluOpType.add)
                    # gamma2 * (dinv * agg) + (beta2 + b2)
                    nc.vector.scalar_tensor_tensor(
                        o_f[:], o_f[:], dv, gb_sb["wg2"][:],
                        mybir.AluOpType.mult, mybir.AluOpType.mult)
                    nc.vector.tensor_tensor(o_f[:], o_f[:], gb_sb["wb2"][:],
                                            mybir.AluOpType.add)
                    o_ln = epi.tile([TP, OUT], f32, tag="oln", name=f"ol_{t}")
                    layernorm(o_ln[:], o_f[:], OUT)
                    nc.sync.dma_start(out=out_d.ap()[t * TP:(t + 1) * TP, :],
                                      in_=o_ln[:])

    nc.compile()
    return nc

# ---------------------------------------------------------------- runner ----


_CACHE = {}


def _get_program(cfg, calls):
    key = (cfg.NT, calls)
    if key not in _CACHE:
        _CACHE[key] = build_program(cfg, calls)
    return _CACHE[key]


def run(inputs, cfg=FULL, trace=False, **kw):
    in_maps, calls = make_in_maps(inputs, cfg)
    nc = _get_program(cfg, calls)
    res = bass_utils.run_bass_kernel_spmd(
        nc, in_maps, core_ids=list(range(cfg.NC)), trace=trace, **kw)
    out = np.concatenate([res.results[c]["out"] for c in range(cfg.NC)],
                         axis=0)[: cfg.n_real]
    return out.astype(np.float32), res


def kernel(**inputs):
    out, _ = run(inputs, FULL)
    return out


# revision 14
# speedup vs baseline: 1.6560x; 1.0030x over previous
"""Trainium2 Bass kernel for nn_MetaSignatureEncoder (GCN encoder with FiLM
signature conditioning), distributed over 8 NeuronCores.

Strategy (graph/data parallel, per the sharding hint):
  - Nodes are padded to NPAD = 8*49*128 = 50176 and sharded contiguously
    across the 8 cores (6272 nodes/core, 49 dst tiles of 128).
  - norm[e] = dinv[src]*dinv[dst] factors out of the message sum, so rows are
    pre-scaled once (h' = dinv * (x @ W)), edges aggregate UNWEIGHTED, and the
    dst factor is applied after aggregation.  Self-loops are a local add.
  - Phase 1: each core computes h' for its shard (PE matmul), AllGather
    replicates the full [NPAD, 512] bf16 table to every core's HBM.
  - Pass 1 (edges): per dst-tile of 128 nodes, dma_gather fetches the h' rows
    of all in-edges (grouped per tile on the host, int16 indices, node space
    split in two segments because gather indices are int16), and the
    segment-sum is done on the TensorEngine: for each chunk of 128 messages a
    one-hot S matrix ([128,128], S[j,d] = 1 iff dst(msg j) == d, built by one
    DVE is_equal against an iota row) is matmul'd with the gathered rows,
    accumulating in PSUM.  The number of 128-message chunks per (tile, seg)
    is baked into the program per problem instance (no padded descriptors).
  - The graph signature s = sum_n relu(agg_sig + b) is reduced over nodes
    with a mask-vector matmul (PE) and AllReduce'd (f32: the fc preacts are
    ~1e4 scale, tanh sign flips forbid bf16 here).
  - gamma/beta FiLM vectors are computed redundantly on every core with f32
    matmuls whose lhsT is s broadcast along the free axis.
  - Encoder: FiLM + relu + LN (bn_stats/bn_aggr) per tile, conv2 matmul via
    PE transpose, dinv scale, AllGather of the [NPAD, 128] bf16 table,
    second edge pass identical in structure, FiLM + LN epilogue, output.

kernel(**inputs) takes the FULL problem inputs and returns the FULL output.
"""
import os
import sys
import math
import numpy as np
import ml_dtypes

sys.path.insert(0, "/opt/trn_rl_repo")

from concourse import bass, bacc, tile, mybir
from concourse import bass_utils

BF16 = ml_dtypes.bfloat16
dt = mybir.dt

# ---------------------------------------------------------------- config ----


class Cfg:
    def __init__(self, NT=49, n_real=50000, n_edges=800000):
        self.NC = 8           # cores
        self.TP = 128         # partitions / dst-tile size
        self.NT = NT          # dst tiles per core
        self.SPLITS = 2       # src-space segments (int16 idx < 32768)
        self.MAXI = 1024      # max dma_gather descriptors per call
        self.IN_CH = 256
        self.HID = 256
        self.OUT = 128
        self.FUSED = self.HID + self.HID   # sig(256) | conv1(256)
        self.KX = self.IN_CH // 128        # K chunks for x matmul
        self.KH = self.HID // 128          # K chunks for conv2 matmul
        self.KA = 3                        # K chunks for augmented fc matmuls
        self.SHARD = self.NT * self.TP
        self.NPAD = self.NC * self.SHARD
        self.NT_A = 20                     # tiles in AllGather half A (17..32: both halves fit int16)
        self.ROWS_A = self.NT_A * self.TP  # 3200 local rows in half A
        # half-table sizes (all cores' A rows; all cores' B rows)
        self.TBL_A = self.NC * self.ROWS_A
        self.TBL_B = self.NPAD - self.TBL_A
        self.n_real = n_real
        self.n_edges = n_edges
        self.LN_EPS = 1e-5


FULL = Cfg()

# ------------------------------------------------------------ host side -----


def _wrap16(vals, nrows=128):
    """dma_gather index layout: idx j at [j % 16, j // 16], replicated to all
    8 q7 core groups (rows 16k+p == row p)."""
    n = vals.shape[0]
    assert n % 16 == 0
    w = vals.reshape(n // 16, 16).T          # [16, n/16]
    return np.tile(w, (nrows // 16, 1))      # [128, n/16]


def _pmaj(vals, TP=128):
    """[NT*TP] -> [TP, NT] partition-major (tile t col, partition p row)."""
    return np.ascontiguousarray(vals.reshape(-1, TP).T)


def preprocess(edge_index, cfg):
    """Integer-only graph preprocessing -> per-core gather structures.

    Per core: a flat list of dma_gather calls, each for one (dst-tile, seg)
    with a compile-time chunk count C_i = ceil(n_i / 128) <= MAXI/128.
    Returns (deg, per_core, calls) where calls[i] = (tile, C_i) is shared
    by all cores (max over cores, so one SPMD program fits all).
    """
    src = np.asarray(edge_index[0], dtype=np.int64)
    dst = np.asarray(edge_index[1], dtype=np.int64)
    deg = np.bincount(src, minlength=cfg.NPAD).astype(np.float32) + 1.0

    SP = cfg.SPLITS
    # Segment = which half of its owner shard a source row lives in.
    # Half A = local rows [0, ROWS_A), half B = [ROWS_A, SHARD).  The
    # AllGather runs as two collectives (A then B), so gathers against
    # table A can start while B is still in flight.
    RA, RB = cfg.ROWS_A, cfg.SHARD - cfg.ROWS_A
    s_core = src // cfg.SHARD
    s_lr = src % cfg.SHARD
    s_seg = (s_lr >= RA).astype(np.int64)
    s_rel = np.where(s_seg == 0, s_core * RA + s_lr,
                     s_core * RB + (s_lr - RA))

    # group per (core, tile, seg)
    per_core_lists = []
    shard_of = dst // cfg.SHARD
    for c in range(cfg.NC):
        m = shard_of == c
        s_rel_c, s_seg_c = s_rel[m], s_seg[m]
        d_c = dst[m] - c * cfg.SHARD
        tile_of = d_c // cfg.TP
        order = np.argsort(tile_of, kind="stable")
        s_rel_c, s_seg_c = s_rel_c[order], s_seg_c[order]
        d_c, tile_of = d_c[order], tile_of[order]
        bounds = np.searchsorted(tile_of, np.arange(cfg.NT + 1))
        tiles = []
        for t in range(cfg.NT):
            sl = slice(bounds[t], bounds[t + 1])
            s_t, g_t = s_rel_c[sl], s_seg_c[sl]
            d_t = d_c[sl] - t * cfg.TP
            segs = []
            for k in range(SP):
                a = g_t == k
                o = np.argsort(s_t[a], kind="stable")
                segs.append((s_t[a][o], d_t[a][o]))
            tiles.append(segs)
        per_core_lists.append(tiles)

    # uniform call structure: per (tile, seg) C = ceil(max_c n / 128),
    # split into multiple calls if C > MAXI/128
    maxc = cfg.MAXI // cfg.TP
    calls = []      # list of (tile, seg, chunk_count)
    for t in range(cfg.NT):
        for k in range(SP):
            nmax = max(len(per_core_lists[c][t][k][0])
                       for c in range(cfg.NC))
            C = max(1, math.ceil(nmax / cfg.TP))
            while C > maxc:
                calls.append((t, k, maxc))
                C -= maxc
            calls.append((t, k, C))

    per_core = []
    for c in range(cfg.NC):
        idx_parts, seg_parts = [], []
        consumed = {}
        for (t, k, C) in calls:
            n_slots = C * cfg.TP
            s_all, d_all = per_core_lists[c][t][k]
            lo = consumed.get((t, k), 0)
            hi = min(lo + n_slots, len(s_all))
            consumed[(t, k)] = hi
            ii = np.zeros(n_slots, np.int64)
            ss = -np.ones(n_slots, np.float32)
            ii[: hi - lo] = s_all[lo:hi]
            ss[: hi - lo] = d_all[lo:hi]
            idx_parts.append(_wrap16(ii).astype(np.int16))
            seg_parts.append(np.ascontiguousarray(
                ss.reshape(C, cfg.TP).T).astype(BF16))
        per_core.append({
            "idx": np.concatenate(idx_parts, axis=1),
            "seg": np.concatenate(seg_parts, axis=1),
        })
    return deg, per_core, tuple(calls)


def make_in_maps(inputs, cfg):
    """Build the per-core input maps for run_bass_kernel_spmd."""
    x = np.asarray(inputs["x"], np.float32)
    deg, per_core, calls = preprocess(np.asarray(inputs["edge_index"]), cfg)

    xp = np.zeros((cfg.NPAD, cfg.IN_CH), np.float32)
    xp[: x.shape[0]] = x

    def chunks(a, k):  # [K*128, N] -> [K, 128, N]
        return np.ascontiguousarray(a.reshape(k, 128, a.shape[1]))

    wf = np.concatenate([np.asarray(inputs["sig_conv_w"], np.float32),
                         np.asarray(inputs["conv1_w"], np.float32)], axis=1)

    def aug(w, b):  # [N, K] weight + [N] bias -> [KA, 128, N] f32 (w.T | b | 0)
        wt = np.asarray(w, np.float32).T
        a = np.zeros((cfg.KA * 128, wt.shape[1]), np.float32)
        a[: wt.shape[0]] = wt
        a[wt.shape[0]] = np.asarray(b, np.float32)
        return chunks(a, cfg.KA)

    shared = {
        "wf": chunks(wf, cfg.KX).astype(BF16),
        "w2": chunks(np.asarray(inputs["conv2_w"], np.float32),
                     cfg.KH).astype(BF16),
        "wg1": aug(inputs["fc1_w"], inputs["fc1_b"]),
        "wb1": aug(inputs["fc2_w"], inputs["fc2_b"]),
        "wg2": aug(inputs["fc3_w"], inputs["fc3_b"]),
        "wb2": aug(inputs["fc4_w"], inputs["fc4_b"]),
        "bsig": np.broadcast_to(np.asarray(inputs["sig_conv_b"], np.float32),
                                (128, cfg.HID)).copy(),
        "b1c": np.broadcast_to(np.asarray(inputs["conv1_b"], np.float32),
                               (128, cfg.HID)).copy(),
        "b2c": np.broadcast_to(np.asarray(inputs["conv2_b"], np.float32),
                               (128, cfg.OUT)).copy(),
        "iota": np.broadcast_to(np.arange(128, dtype=np.float32),
                                (128, 128)).astype(BF16).copy(),
        "ident": np.eye(128, dtype=np.float32).astype(BF16),
    }

    in_maps = []
    node_ids = np.arange(cfg.SHARD)
    for c in range(cfg.NC):
        sl = slice(c * cfg.SHARD, (c + 1) * cfg.SHARD)
        gids = node_ids + c * cfg.SHARD
        m = dict(shared)
        m["xT"] = chunks(np.ascontiguousarray(xp[sl].T), cfg.KX).astype(BF16)
        m["deg"] = _pmaj(deg[sl]).copy()
        m["sigmask"] = _pmaj((gids < cfg.n_real).astype(np.float32)).astype(BF16)
        m.update(per_core[c])
        in_maps.append(m)
    return in_maps, calls

# --------------------------------------------------------------- builder ----


def build_program(cfg, calls):
    nc = bacc.Bacc("TRN2", target_bir_lowering=False, debug=False,
                   num_devices=cfg.NC,
                   num_swdge_queues=4)
    f32, bf16, i16 = dt.float32, dt.bfloat16, dt.int16
    TP, NT = cfg.TP, cfg.NT
    HID, OUT, FUSED = cfg.HID, cfg.OUT, cfg.FUSED
    SP = cfg.SPLITS
    TOTC = sum(C for (_, _, C) in calls)

    def inp(name, shape, dtype):
        return nc.dram_tensor(name, shape, dtype, kind="ExternalInput")

    xT_d = inp("xT", [cfg.KX, TP, cfg.SHARD], bf16)
    wf_d = inp("wf", [cfg.KX, TP, FUSED], bf16)
    w2_d = inp("w2", [cfg.KH, TP, OUT], bf16)
    wg1_d = inp("wg1", [cfg.KA, TP, HID], f32)
    wb1_d = inp("wb1", [cfg.KA, TP, HID], f32)
    wg2_d = inp("wg2", [cfg.KA, TP, OUT], f32)
    wb2_d = inp("wb2", [cfg.KA, TP, OUT], f32)
    bsig_d = inp("bsig", [TP, HID], f32)
    b1c_d = inp("b1c", [TP, HID], f32)
    b2c_d = inp("b2c", [TP, OUT], f32)
    iota_d = inp("iota", [TP, TP], bf16)
    ident_d = inp("ident", [TP, TP], bf16)
    deg_d = inp("deg", [TP, NT], f32)
    mask_d = inp("sigmask", [TP, NT], bf16)
    idx_d = inp("idx", [TP, TOTC * 8], i16)
    seg_d = inp("seg", [TP, TOTC], bf16)

    out_d = nc.dram_tensor("out", [cfg.SHARD, OUT], f32, kind="ExternalOutput")

    # internal DRAM (collective bounce buffers, split in two halves so the
    # second AllGather can overlap gathers against the first half)
    RA, RB = cfg.ROWS_A, cfg.SHARD - cfg.ROWS_A
    hshA_d = nc.dram_tensor("hshA", [RA, FUSED], bf16)
    hshB_d = nc.dram_tensor("hshB", [RB, FUSED], bf16)
    hfullA_d = nc.dram_tensor("hfullA", [cfg.TBL_A, FUSED], bf16,
                              addr_space="Shared")
    hfullB_d = nc.dram_tensor("hfullB", [cfg.TBL_B, FUSED], bf16,
                              addr_space="Shared")
    tshA_d = nc.dram_tensor("tshA", [RA, OUT], bf16)
    tshB_d = nc.dram_tensor("tshB", [RB, OUT], bf16)
    tfullA_d = nc.dram_tensor("tfullA", [cfg.TBL_A, OUT], bf16,
                              addr_space="Shared")
    tfullB_d = nc.dram_tensor("tfullB", [cfg.TBL_B, OUT], bf16,
                              addr_space="Shared")
    stage1_d = nc.dram_tensor("stage1", [cfg.SHARD, FUSED], f32)
    stage2_d = nc.dram_tensor("stage2", [cfg.SHARD, OUT], f32)
    sin_d = nc.dram_tensor("sin", [1, HID], f32)
    sout_d = nc.dram_tensor("sout", [1, HID], f32, addr_space="Shared")

    rg = [list(range(cfg.NC))]

    # per-call cumulative offsets into idx/seg
    call_off = []
    o = 0
    for (_, _, C) in calls:
        call_off.append(o)
        o += C

    # group calls per tile (they are consecutive by construction)
    tile_calls = {t: [] for t in range(NT)}
    for i, (t, k, C) in enumerate(calls):
        tile_calls[t].append((i, k, C))

    with tile.TileContext(nc) as tc:
        with (
            tc.tile_pool(name="const", bufs=1) as const,
            tc.tile_pool(name="persist", bufs=1) as persist,
            tc.tile_pool(name="xload", bufs=3) as xload,
            tc.tile_pool(name="gat", bufs=4) as gat,
            tc.tile_pool(name="sbuild", bufs=4) as sbuild,
            tc.tile_pool(name="epi", bufs=3) as epi,
            tc.tile_pool(name="small", bufs=4) as small,
            tc.tile_pool(name="one", bufs=1) as one,
            tc.tile_pool(name="ps_big", bufs=2, space="PSUM") as ps_big,
            tc.tile_pool(name="ps_sig", bufs=1, space="PSUM") as ps_sig,
            tc.tile_pool(name="ps_sm", bufs=3, space="PSUM") as ps_sm,
        ):
            # ---- load constants -------------------------------------------
            wf_sb = const.tile([TP, cfg.KX, FUSED], bf16)
            w2_sb = const.tile([TP, cfg.KH, OUT], bf16)
            nc.sync.dma_start(out=wf_sb[:], in_=wf_d.ap().transpose([1, 0, 2]))
            nc.sync.dma_start(out=w2_sb[:], in_=w2_d.ap().transpose([1, 0, 2]))
            fc_sb = {}
            for nm, d, width in (("wg1", wg1_d, HID), ("wb1", wb1_d, HID),
                                 ("wg2", wg2_d, OUT), ("wb2", wb2_d, OUT)):
                t_ = const.tile([TP, cfg.KA, width], f32, name=nm)
                nc.sync.dma_start(out=t_[:], in_=d.ap().transpose([1, 0, 2]))
                fc_sb[nm] = t_
            bsig_sb = const.tile([TP, HID], f32)
            b1c_sb = const.tile([TP, HID], f32)
            b2c_sb = const.tile([TP, OUT], f32)
            iota_sb = const.tile([TP, TP], bf16)
            ident_sb = const.tile([TP, TP], bf16)
            deg_sb = const.tile([TP, NT], f32)
            mask_sb = const.tile([TP, NT], bf16)
            idx_sb = const.tile([TP, TOTC * 8], i16)
            seg_sb = const.tile([TP, TOTC], bf16)
            for t_, d in ((bsig_sb, bsig_d), (b1c_sb, b1c_d), (b2c_sb, b2c_d),
                          (iota_sb, iota_d), (ident_sb, ident_d),
                          (deg_sb, deg_d), (mask_sb, mask_d),
                          (idx_sb, idx_d), (seg_sb, seg_d)):
                nc.sync.dma_start(out=t_[:], in_=d.ap())

            eps_sb = const.tile([TP, 1], f32)
            nc.vector.memset(eps_sb[:], cfg.LN_EPS)

            # dinv = 1/sqrt(deg)
            dinv_sb = const.tile([TP, NT], f32)
            nc.scalar.sqrt(dinv_sb[:], deg_sb[:])
            nc.vector.reciprocal(dinv_sb[:], dinv_sb[:])

            # persistent per-shard state
            hsh_sb = persist.tile([TP, NT, FUSED], bf16)    # h' shard
            c1agg_sb = persist.tile([TP, NT, HID], bf16)    # conv1 aggregate
            tp_sb = persist.tile([TP, NT, OUT], bf16)       # t' shard

            # ---- phase 1: h' = dinv * (x @ [Wsig|W1]), allgather ----------
            NT_A = cfg.NT_A
            with nc.named_scope("phase1"):
                for t in range(NT):
                    xt = xload.tile([TP, cfg.KX, TP], bf16, tag="xt")
                    for k in range(cfg.KX):
                        nc.sync.dma_start(
                            out=xt[:, k, :],
                            in_=xT_d.ap()[k, :, t * TP:(t + 1) * TP])
                    ps = ps_big.tile([TP, FUSED], f32, tag="mm")
                    for k in range(cfg.KX):
                        nc.tensor.matmul(ps[:], xt[:, k, :], wf_sb[:, k, :],
                                         start=(k == 0), stop=(k == cfg.KX - 1))
                    nc.scalar.activation(hsh_sb[:, t, :], ps[:],
                                         mybir.ActivationFunctionType.Copy,
                                         scale=dinv_sb[:, t:t + 1])
                    if t < NT_A:
                        dst_ap = hshA_d.ap()[t * TP:(t + 1) * TP, :]
                    else:
                        dst_ap = hshB_d.ap()[(t - NT_A) * TP:
                                             (t - NT_A + 1) * TP, :]
                    nc.sync.dma_start(out=dst_ap, in_=hsh_sb[:, t, :])
                    if t == NT_A - 1:
                        nc.gpsimd.collective_compute(
                            "AllGather", mybir.AluOpType.bypass,
                            replica_groups=rg,
                            ins=[hshA_d.ap().opt()],
                            outs=[hfullA_d.ap().opt()])
                nc.gpsimd.collective_compute(
                    "AllGather", mybir.AluOpType.bypass, replica_groups=rg,
                    ins=[hshB_d.ap().opt()], outs=[hfullB_d.ap().opt()])

            # ---- edge aggregation machinery -------------------------------
            s_ps = ps_sig.tile([1, HID], f32)
            qctr = [0]

            def seg_gather(t, k, tblA, tblB, width, ps_pool, tag):
                """gather + segsum-matmul over segment k of dst-tile t."""
                ps = ps_pool.tile([TP, width], f32, tag=tag,
                                  name=f"ps{tag}_{t}_{k}")
                ncalls = [c for c in tile_calls[t] if c[1] == k]
                total = sum(C for (_, _, C) in ncalls)
                done = 0
                for (ci, kk, C) in ncalls:
                    table = (tblA if kk == 0 else tblB).ap()
                    off = call_off[ci]
                    g = gat.tile([TP, cfg.MAXI // TP, width],
                                 bf16, tag=f"g{width}", name=f"g_{t}_{ci}")
                    nc.gpsimd.dma_gather(
                        out_ap=g[:, :C, :],
                        in_ap=table,
                        idxs_ap=idx_sb[:, off * 8:(off + C) * 8],
                        num_idxs=C * TP,
                        num_idxs_reg=C * TP,
                        elem_size=width,
                        queue_num=qctr[0] % 4,
                    )
                    qctr[0] += 1
                    S = sbuild.tile([TP, cfg.MAXI // TP, TP], bf16, tag="S",
                                    name=f"S_{t}_{ci}")
                    seg_col = seg_sb[:, off:off + C]
                    nc.vector.tensor_tensor(
                        S[:, :C, :],
                        seg_col.unsqueeze(2).to_broadcast((TP, C, TP)),
                        iota_sb[:].unsqueeze(1).to_broadcast((TP, C, TP)),
                        mybir.AluOpType.is_equal)
                    for j in range(C):
                        done += 1
                        nc.tensor.matmul(ps[:], S[:, j, :], g[:, j, :],
                                         start=(done == 1),
                                         stop=(done == total))
                return ps

            # ---- pass 1: edge aggregation over h' -------------------------
            # A phase: aggregate half-A edges for every tile, park the
            # partial sums in DRAM (stage1).  Runs as soon as the A
            # AllGather lands, overlapping the B AllGather.
            with nc.named_scope("pass1a"):
                for t in range(NT):
                    ps = seg_gather(t, 0, hfullA_d, hfullB_d, FUSED,
                                    ps_big, "mm")
                    stg = epi.tile([TP, FUSED], f32, tag="stg",
                                   name=f"stg_{t}")
                    nc.vector.tensor_copy(stg[:], ps[:])
                    nc.sync.dma_start(
                        out=stage1_d.ap()[t * TP:(t + 1) * TP, :],
                        in_=stg[:])
            with nc.named_scope("pass1b"):
                for t in range(NT):
                    ps = seg_gather(t, 1, hfullA_d, hfullB_d, FUSED,
                                    ps_big, "mm")
                    stg = epi.tile([TP, FUSED], f32, tag="stg",
                                   name=f"stgi_{t}")
                    nc.sync.dma_start(
                        out=stg[:],
                        in_=stage1_d.ap()[t * TP:(t + 1) * TP, :])
                    dv = dinv_sb[:, t:t + 1]
                    tot_f = epi.tile([TP, FUSED], f32, tag="totf",
                                     name=f"tot_{t}")
                    nc.vector.tensor_tensor(tot_f[:], ps[:], stg[:],
                                            mybir.AluOpType.add)
                    # sig half: relu((psum + selfloop)*dinv + bsig); mask-sum
                    sig_f = epi.tile([TP, HID], f32, tag="sigf")
                    nc.vector.tensor_tensor(sig_f[:], tot_f[:, :HID],
                                            hsh_sb[:, t, :HID],
                                            mybir.AluOpType.add)
                    nc.vector.scalar_tensor_tensor(
                        sig_f[:], sig_f[:], dv, bsig_sb[:],
                        mybir.AluOpType.mult, mybir.AluOpType.add)
                    sig_b = epi.tile([TP, HID], bf16, tag="sigb")
                    nc.scalar.activation(sig_b[:], sig_f[:],
                                         mybir.ActivationFunctionType.Relu)
                    nc.tensor.matmul(s_ps[:], mask_sb[:, t:t + 1], sig_b[:],
                                     start=(t == 0), stop=(t == NT - 1))
                    # conv1 half: (psum + selfloop)*dinv -> bf16
                    c1_f = epi.tile([TP, HID], f32, tag="c1f")
                    nc.vector.tensor_tensor(c1_f[:], tot_f[:, HID:],
                                            hsh_sb[:, t, HID:],
                                            mybir.AluOpType.add)
                    nc.scalar.activation(c1agg_sb[:, t, :], c1_f[:],
                                         mybir.ActivationFunctionType.Copy,
                                         scale=dv)

            # ---- signature allreduce + gamma/beta -------------------------
            with nc.named_scope("signature"):
                s_sb = one.tile([1, HID], f32)
                nc.scalar.copy(s_sb[:], s_ps[:])
                nc.sync.dma_start(out=sin_d.ap(), in_=s_sb[:])
                nc.gpsimd.collective_compute(
                    "AllReduce", mybir.AluOpType.add, replica_groups=rg,
                    ins=[sin_d.ap().opt()], outs=[sout_d.ap().opt()])

                # load s as columns [128, KA]; aug row (=1.0) in col KA-1
                s_col = one.tile([TP, cfg.KA], f32)
                nc.vector.memset(s_col[:], 0.0)
                nc.vector.memset(s_col[0:1, cfg.KA - 1:cfg.KA], 1.0)
                nc.sync.dma_start(
                    out=s_col[:, 0:2],
                    in_=sout_d.ap().rearrange("o (c p) -> (o c) p", p=TP)
                        .transpose([1, 0]))
                s_rep = one.tile([TP, cfg.KA, TP], f32)
                for c in range(cfg.KA):
                    nc.vector.tensor_copy(
                        s_rep[:, c, :],
                        s_col[:, c:c + 1].to_broadcast((TP, TP)))

                gb_sb = {}
                for nm, width in (("wg1", HID), ("wb1", HID),
                                  ("wg2", OUT), ("wb2", OUT)):
                    ps_fc = ps_sm.tile([TP, width], f32, tag="sm", name=nm)
                    for c in range(cfg.KA):
                        nc.tensor.matmul(ps_fc[:], s_rep[:, c, :],
                                         fc_sb[nm][:, c, :],
                                         start=(c == 0), stop=(c == cfg.KA - 1))
                    gb = one.tile([TP, width], f32, name=f"gb_{nm}", tag=nm)
                    nc.scalar.activation(gb[:], ps_fc[:],
                                         mybir.ActivationFunctionType.Tanh)
                    gb_sb[nm] = gb
                # beta + conv bias
                nc.vector.tensor_tensor(gb_sb["wb1"][:], gb_sb["wb1"][:],
                                        b1c_sb[:], mybir.AluOpType.add)
                nc.vector.tensor_tensor(gb_sb["wb2"][:], gb_sb["wb2"][:],
                                        b2c_sb[:], mybir.AluOpType.add)

            # ---- encoder local: FiLM + relu + LN + conv2 matmul -----------
            def layernorm(dst_ap, src_ap, width):
                st6 = small.tile([TP, 6], f32, tag="st6", name="st6")
                mv = small.tile([TP, 2], f32, tag="mv", name="mv")
                nc.vector.bn_stats(st6[:], src_ap)
                nc.vector.bn_aggr(mv[:], st6[:])
                std = small.tile([TP, 1], f32, tag="std", name="std")
                nc.scalar.activation(std[:], mv[:, 1:2],
                                     mybir.ActivationFunctionType.Sqrt,
                                     bias=eps_sb[:, 0:1])
                rstd = small.tile([TP, 1], f32, tag="rstd", name="rstd")
                nc.vector.reciprocal(rstd[:], std[:])
                nmr = small.tile([TP, 1], f32, tag="nmr", name="nmr")
                nc.vector.scalar_tensor_tensor(
                    nmr[:], mv[:, 0:1], -1.0, rstd[:],
                    mybir.AluOpType.mult, mybir.AluOpType.mult)
                nc.scalar.activation(dst_ap, src_ap,
                                     mybir.ActivationFunctionType.Identity,
                                     bias=nmr[:, 0:1], scale=rstd[:, 0:1])

            with nc.named_scope("encoder_local"):
                for t in range(NT):
                    h_f = epi.tile([TP, HID], f32, tag="hf", name=f"h_{t}")
                    nc.vector.tensor_tensor(h_f[:], c1agg_sb[:, t, :],
                                            gb_sb["wg1"][:],
                                            mybir.AluOpType.mult)
                    nc.vector.tensor_tensor(h_f[:], h_f[:], gb_sb["wb1"][:],
                                            mybir.AluOpType.add)
                    nc.scalar.activation(h_f[:], h_f[:],
                                         mybir.ActivationFunctionType.Relu)
                    h1 = epi.tile([TP, HID], bf16, tag="h1", name=f"h1_{t}")
                    layernorm(h1[:], h_f[:], HID)
                    # transpose h1 tile and matmul with w2
                    h1T = epi.tile([TP, cfg.KH, TP], bf16, tag="h1T",
                                   name=f"h1T_{t}")
                    for c in range(cfg.KH):
                        ps_t = ps_sm.tile([TP, TP], bf16, tag="sm",
                                          name=f"tr_{t}_{c}")
                        nc.tensor.transpose(ps_t[:],
                                            h1[:, c * TP:(c + 1) * TP],
                                            ident_sb[:])
                        nc.vector.tensor_copy(h1T[:, c, :], ps_t[:])
                    ps2 = ps_sm.tile([TP, OUT], f32, tag="sm", name=f"w2_{t}")
                    for c in range(cfg.KH):
                        nc.tensor.matmul(ps2[:], h1T[:, c, :], w2_sb[:, c, :],
                                         start=(c == 0), stop=(c == cfg.KH - 1))
                    nc.scalar.activation(tp_sb[:, t, :], ps2[:],
                                         mybir.ActivationFunctionType.Copy,
                                         scale=dinv_sb[:, t:t + 1])
                    if t < NT_A:
                        dst_ap = tshA_d.ap()[t * TP:(t + 1) * TP, :]
                    else:
                        dst_ap = tshB_d.ap()[(t - NT_A) * TP:
                                             (t - NT_A + 1) * TP, :]
                    nc.sync.dma_start(out=dst_ap, in_=tp_sb[:, t, :])
                    if t == NT_A - 1:
                        nc.gpsimd.collective_compute(
                            "AllGather", mybir.AluOpType.bypass,
                            replica_groups=rg,
                            ins=[tshA_d.ap().opt()],
                            outs=[tfullA_d.ap().opt()])
                nc.gpsimd.collective_compute(
                    "AllGather", mybir.AluOpType.bypass, replica_groups=rg,
                    ins=[tshB_d.ap().opt()], outs=[tfullB_d.ap().opt()])

            # ---- pass 2: edge aggregation over t' -------------------------
            with nc.named_scope("pass2a"):
                for t in range(NT):
                    ps = seg_gather(t, 0, tfullA_d, tfullB_d, OUT,
                                    ps_big, "mm2")
                    stg = epi.tile([TP, OUT], f32, tag="stg2",
                                   name=f"st2_{t}")
                    nc.vector.tensor_copy(stg[:], ps[:])
                    nc.sync.dma_start(
                        out=stage2_d.ap()[t * TP:(t + 1) * TP, :],
                        in_=stg[:])
            with nc.named_scope("pass2b"):
                for t in range(NT):
                    ps = seg_gather(t, 1, tfullA_d, tfullB_d, OUT,
                                    ps_big, "mm2")
                    stg = epi.tile([TP, OUT], f32, tag="stg2",
                                   name=f"st2i_{t}")
                    nc.sync.dma_start(
                        out=stg[:],
                        in_=stage2_d.ap()[t * TP:(t + 1) * TP, :])
                    dv = dinv_sb[:, t:t + 1]
                    o_f = epi.tile([TP, OUT], f32, tag="of", name=f"o_{t}")
                    nc.vector.tensor_tensor(o_f[:], ps[:], stg[:],
                                            mybir.AluOpType.add)
                    nc.vector.tensor_tensor(o_f[:], o_f[:],
                                            tp_sb[:, t, :],
                                            mybir.AluOpType.add)
                    # gamma2 * (dinv * agg) + (beta2 + b2)
                    nc.vector.scalar_tensor_tensor(
                        o_f[:], o_f[:], dv, gb_sb["wg2"][:],
                        mybir.AluOpType.mult, mybir.AluOpType.mult)
                    nc.vector.tensor_tensor(o_f[:], o_f[:], gb_sb["wb2"][:],
                                            mybir.AluOpType.add)
                    o_ln = epi.tile([TP, OUT], f32, tag="oln", name=f"ol_{t}")
                    layernorm(o_ln[:], o_f[:], OUT)
                    nc.sync.dma_start(out=out_d.ap()[t * TP:(t + 1) * TP, :],
                                      in_=o_ln[:])

    nc.compile()
    return nc

# ---------------------------------------------------------------- runner ----


_CACHE = {}


def _get_program(cfg, calls):
    key = (cfg.NT, calls)
    if key not in _CACHE:
        _CACHE[key] = build_program(cfg, calls)
    return _CACHE[key]


def run(inputs, cfg=FULL, trace=False, **kw):
    in_maps, calls = make_in_maps(inputs, cfg)
    nc = _get_program(cfg, calls)
    res = bass_utils.run_bass_kernel_spmd(
        nc, in_maps, core_ids=list(range(cfg.NC)), trace=trace, **kw)
    out = np.concatenate([res.results[c]["out"] for c in range(cfg.NC)],
                         axis=0)[: cfg.n_real]
    return out.astype(np.float32), res


def kernel(**inputs):
    out, _ = run(inputs, FULL)
    return out
